# revision 1
# baseline (speedup 1.0000x reference)
"""GPT2 eager causal attention (B=2, S=2048, D=1024, H=16, HD=64) on 8 TRN2 NeuronCores.

Sharding (data + head/tensor parallel, per the problem's hint):
  core c -> (batch b = c//4, head-group g = c%4) -- 4 heads per group.

Per-core pipeline (all layouts chosen so no score-matrix transpose is ever needed):
  1. x[b] transposed on PE -> xT [d, s]                      (d on partitions)
  2. QT,KT = wq/wk^T @ xT  -> [256, S] transposed layouts    (head-dim on partitions)
     V     = xT^T @ wv_ext -> [S, 260] natural, with a ones-column per head
  3. scores^T tiles ST[k, q] = KT_h^T-slices @ QT_h-slices   (k on partitions)
     exp on ScalarE with the 1/sqrt(64) scale folded in; causal masking via
     precomputed mask tiles on diagonal blocks only
     OT[d, q] += V^T-slices @ ST_exp : the ones-column makes row 64 the softmax
     denominator for free; normalize OT by its reciprocal (gpsimd broadcast)
  4. c_proj partial = OT^T-slices @ w_proj[group rows]
  5. ReduceScatter(add) over each 4-core (same-batch) group; each core emits its
     [512, 1024] token slice; host reassembles the [2, 2048, 1024] output.

Matmuls run as float32r (full-rate on PE for free dim >= 256, ~tf32 precision),
fp32 accumulation in PSUM, all storage fp32.
"""
from contextlib import ExitStack

import ml_dtypes
import numpy as np

import concourse.bacc as bacc
import concourse.mybir as mybir
import concourse.tile as tile
from concourse.bass_utils import run_bass_kernel_spmd

F32 = mybir.dt.float32
F32R = mybir.dt.float32r
BF16 = mybir.dt.bfloat16

B, S, D, H, HD = 2, 2048, 1024, 16, 64
N_CORES = 8
HG = 4               # heads per group
DG = HG * HD         # 256 q/k channels per group
VW = HG * (HD + 1)   # 260: 64 v-cols + 1 ones-col per head
NK = D // 128        # 8 contraction tiles over d
NS = S // 128        # 16 token tiles
CH = 512             # q-chunk (one PSUM bank of fp32)
NCH = S // CH        # 4
NRT = DG // 128      # 2 channel row-tiles per group


def _build(has_bv: bool, has_bp: bool, has_bqk: bool = False, tail: str = "rs", phases: int = 99):
    nc = bacc.Bacc("TRN2", target_bir_lowering=False, debug=False, num_devices=N_CORES)

    x_d = nc.dram_tensor("x", [S, D], BF16, kind="ExternalInput").ap()
    wq_d = nc.dram_tensor("wq", [D, DG], BF16, kind="ExternalInput").ap()
    wk_d = nc.dram_tensor("wk", [D, DG], BF16, kind="ExternalInput").ap()
    wv_d = nc.dram_tensor("wv", [D, VW], BF16, kind="ExternalInput").ap()
    wp_d = nc.dram_tensor("wp", [DG, D], BF16, kind="ExternalInput").ap()
    bq_d = nc.dram_tensor("bq", [DG, 1], F32, kind="ExternalInput").ap()
    bk_d = nc.dram_tensor("bk", [DG, 1], F32, kind="ExternalInput").ap()
    bv_d = nc.dram_tensor("bv", [DG, 1], F32, kind="ExternalInput").ap()
    bp_d = nc.dram_tensor("bp", [128, D], F32, kind="ExternalInput").ap()
    mk_d = nc.dram_tensor("masks", [128, 128], BF16, kind="ExternalInput").ap()
    if tail == "rs":
        out_d = nc.dram_tensor("out", [CH, D], F32, kind="ExternalOutput").ap()
    else:  # debug: emit the full per-core partial
        out_d = nc.dram_tensor("out", [S, D], F32, kind="ExternalOutput").ap()

    EXP = mybir.ActivationFunctionType.Exp
    IDENT = mybir.ActivationFunctionType.Identity

    with ExitStack() as ctx:
        tc = ctx.enter_context(tile.TileContext(nc))
        wpool = ctx.enter_context(tc.tile_pool(name="w", bufs=1))
        big = ctx.enter_context(tc.tile_pool(name="big", bufs=8))
        qkvp = ctx.enter_context(tc.tile_pool(name="qkv", bufs=1))
        stp = ctx.enter_context(tc.tile_pool(name="stx", bufs=6))
        nrm = ctx.enter_context(tc.tile_pool(name="nrm", bufs=2))
        outp = ctx.enter_context(tc.tile_pool(name="outp", bufs=3))
        ps_mm = ctx.enter_context(tc.tile_pool(name="psmm", bufs=3, space="PSUM"))
        ps_st = ctx.enter_context(tc.tile_pool(name="psst", bufs=3, space="PSUM"))
        ps_ot = ctx.enter_context(tc.tile_pool(name="psot", bufs=2, space="PSUM"))
        dram = ctx.enter_context(tc.tile_pool(name="dram", bufs=1, space="DRAM"))

        # ---- constants / weights -> SBUF
        wq_sb = wpool.tile([128, NK * DG], BF16)
        wk_sb = wpool.tile([128, NK * DG], BF16)
        wv_sb = wpool.tile([128, NK * VW], BF16)
        wp_sb = wpool.tile([128, NRT * D], BF16)
        mk_sb = wpool.tile([128, 128], BF16)
        on_sb = wpool.tile([1, 64], F32)
        bq_sb = wpool.tile([128, NRT], F32)
        bk_sb = wpool.tile([128, NRT], F32)
        bv_sb = wpool.tile([128, NRT], F32) if has_bv else None
        bp_sb = wpool.tile([128, D], F32) if has_bp else None
        for kt in range(NK):
            nc.sync.dma_start(wq_sb[:, kt * DG:(kt + 1) * DG], wq_d[kt * 128:(kt + 1) * 128, :])
            nc.sync.dma_start(wk_sb[:, kt * DG:(kt + 1) * DG], wk_d[kt * 128:(kt + 1) * 128, :])
            nc.sync.dma_start(wv_sb[:, kt * VW:(kt + 1) * VW], wv_d[kt * 128:(kt + 1) * 128, :])
        for rt in range(NRT):
            nc.sync.dma_start(wp_sb[:, rt * D:(rt + 1) * D], wp_d[rt * 128:(rt + 1) * 128, :])
            nc.sync.dma_start(bq_sb[:, rt:rt + 1], bq_d[rt * 128:(rt + 1) * 128, :])
            nc.sync.dma_start(bk_sb[:, rt:rt + 1], bk_d[rt * 128:(rt + 1) * 128, :])
            if has_bv:
                nc.sync.dma_start(bv_sb[:, rt:rt + 1], bv_d[rt * 128:(rt + 1) * 128, :])
        if has_bp:
            nc.sync.dma_start(bp_sb[:], bp_d[:])
        nc.sync.dma_start(mk_sb[:], mk_d[:])
        nc.vector.memset(on_sb[:], 1.0)

        # ---- phase 1: xT strips [128 d, S] via transpose-DMA (bf16 xbar path),
        # split into column chunks; weights were enqueued first so QKV can
        # start as soon as the sq=0 chunks land
        xT = []
        for dt in range(NK):
            t = big.tile([128, S], BF16, tag="bigslot", name=f"xT{dt}")
            xT.append(t)
        for sq in range(4):
            for dt in range(NK):
                nc.sync.dma_start_transpose(
                    xT[dt][:, sq * CH:(sq + 1) * CH],
                    x_d[sq * CH:(sq + 1) * CH, dt * 128:(dt + 1) * 128],
                )


        # ---- phase 2: QT/KT [256, S] (as 2 tiles of [128, S]) and V strips
        QT, KT = [], []
        for store, w_sb, b_sb, nm in ((QT, wq_sb, bq_sb, "q"), (KT, wk_sb, bk_sb, "k")):
            for rt in range(NRT):
                dst = qkvp.tile([128, S], BF16, tag=f"{nm}t{rt}", name=f"{nm}T{rt}")
                store.append(dst)
                for ch in range(NCH):
                    ps = ps_mm.tile([128, CH], F32, tag="ps", name=f"ps{nm}{rt}_{ch}")
                    for kt in range(NK):
                        nc.tensor.matmul(
                            ps[:],
                            (w_sb[:, kt * DG + rt * 128: kt * DG + (rt + 1) * 128]),
                            (xT[kt][:, ch * CH:(ch + 1) * CH]),
                            start=(kt == 0), stop=(kt == NK - 1),
                        )
                    if has_bqk:
                        nc.scalar.activation(
                            dst[:, ch * CH:(ch + 1) * CH], ps[:], IDENT,
                            bias=b_sb[:, rt:rt + 1],
                        )
                    else:
                        nc.vector.tensor_copy(dst[:, ch * CH:(ch + 1) * CH], ps[:])
        V = []
        for st in range(NS):
            vt = qkvp.tile([128, VW], BF16, tag=f"v{st}", name=f"v{st}")
            ps = ps_mm.tile([128, CH], F32, tag="ps", name=f"psv{st}")
            for kt in range(NK):
                nc.tensor.matmul(
                    ps[:, :VW],
                    (xT[kt][:, st * 128:(st + 1) * 128]),
                    (wv_sb[:, kt * VW:(kt + 1) * VW]),
                    start=(kt == 0), stop=(kt == NK - 1),
                )
            nc.vector.tensor_copy(vt[:], ps[:, :VW])
            for hl in range(HG):
                ones_col = vt[:, hl * (HD + 1) + HD: (hl + 1) * (HD + 1)].bitcast(mybir.dt.uint16)
                nc.vector.memset(ones_col, 0x3F80)  # bits of bf16 1.0
            V.append(vt)

        # ---- phase 3: attention + c_proj, chunk by chunk
        OT = []
        for i in range(NRT):
            t = big.tile([128, S], BF16, tag="bigslot", name=f"OT{i}")
            OT.append(t)
        partials = []
        for ch in range(NCH):
            pt = dram.tile([CH, D], F32, tag=f"partial{ch}", name=f"partial{ch}")
            partials.append(pt)
        rs_outs = []

        def emit_rs(ch):
            # rank r of the quad receives tokens [512*ch + 128*r, +128)
            rs_c = dram.tile([128, D], F32, tag=f"rs{ch}", name=f"rs_out{ch}")
            nc.gpsimd.collective_compute(
                "ReduceScatter",
                mybir.AluOpType.add,
                replica_groups=[[0, 1, 2, 3], [4, 5, 6, 7]],
                ins=[partials[ch].opt()],
                outs=[rs_c.opt()],
            )
            rs_outs.append((ch, rs_c))
        for ch in range(NCH):
            nkt = 4 * (ch + 1)
            for hl in range(HG):
                qt = QT[hl // 2]
                ktile = KT[hl // 2]
                off = 64 * (hl % 2)
                ot_ps = ps_ot.tile([65, CH], F32, tag="ot", name=f"ot{ch}_{hl}")
                for kt in range(nkt):
                    st_ps = ps_st.tile([128, CH], F32, tag="st", name=f"st{ch}_{hl}_{kt}")
                    nc.tensor.matmul(
                        st_ps[:],
                        (ktile[off:off + 64, kt * 128:(kt + 1) * 128]),
                        (qt[off:off + 64, ch * CH:(ch + 1) * CH]),
                        start=True, stop=True,
                    )
                    st_sb = stp.tile([128, CH], BF16, tag="stsb", name=f"se{ch}_{hl}_{kt}")
                    d = kt - 4 * ch
                    if d < 0:
                        nc.scalar.activation(st_sb[:], st_ps[:], EXP, scale=0.125)
                    else:
                        # diagonal strip: exp only the valid suffix, zero the
                        # prefix, triangular-mask the 128-wide diagonal block
                        if d > 0:
                            zc = st_sb[:, 0:d * 128].bitcast(mybir.dt.uint16)
                            nc.vector.memset(zc, 0)
                        nc.scalar.activation(st_sb[:, d * 128:], st_ps[:, d * 128:], EXP, scale=0.125)
                        nc.vector.tensor_mul(
                            st_sb[:, d * 128:(d + 1) * 128],
                            st_sb[:, d * 128:(d + 1) * 128],
                            mk_sb[:, 0:128],
                        )
                    nc.tensor.matmul(
                        ot_ps[:],
                        (V[kt][:, hl * (HD + 1):(hl + 1) * (HD + 1)]),
                        (st_sb[:]),
                        start=(kt == 0), stop=(kt == nkt - 1),
                    )
                den = nrm.tile([1, CH], F32, tag="den", name=f"den{ch}_{hl}")
                nc.vector.tensor_copy(den[:], ot_ps[64:65, :])
                rden = nrm.tile([1, CH], F32, tag="rden", name=f"rden{ch}_{hl}")
                nc.vector.reciprocal_approx_fast(rden[:], den[:])
                # rank-1 PE matmul broadcasts the reciprocal row to 64
                # partitions (keeps gpsimd free for the collectives)
                rbc_ps = ps_mm.tile([64, CH], F32, tag="ps", name=f"rbc{ch}_{hl}")
                nc.tensor.matmul(rbc_ps[:], on_sb[:], rden[:], start=True, stop=True)
                ot_sb = nrm.tile([64, CH], BF16, tag="otsb", name=f"otsb{ch}_{hl}")
                nc.vector.tensor_copy(ot_sb[:], ot_ps[0:64, :])
                dst = OT[hl // 2][off:off + 64, ch * CH:(ch + 1) * CH]
                nc.vector.tensor_mul(dst, ot_sb[:], rbc_ps[:])
                if has_bv:
                    nc.vector.tensor_scalar_add(dst, dst, bv_sb[off:off + 64, hl // 2: hl // 2 + 1])
            # c_proj for this chunk's tokens
            for stl in range(4):
                tok = ch * CH + stl * 128
                for n in range(NRT):
                    po = ps_mm.tile([128, CH], F32, tag="ps", name=f"po{ch}_{stl}_{n}")
                    for k2 in range(NRT):
                        nc.tensor.matmul(
                            po[:],
                            (OT[k2][:, tok:tok + 128]),
                            (wp_sb[:, k2 * D + n * CH: k2 * D + (n + 1) * CH]),
                            start=(k2 == 0), stop=(k2 == NRT - 1),
                        )
                    ob = outp.tile([128, CH], F32, tag="ob", name=f"ob{ch}_{stl}_{n}")
                    if has_bp:
                        nc.vector.tensor_add(ob[:], po[:], bp_sb[:, n * CH:(n + 1) * CH])
                    else:
                        nc.vector.tensor_copy(ob[:], po[:])
                    if tail == "rs":
                        nc.sync.dma_start(partials[ch][stl * 128:(stl + 1) * 128, n * CH:(n + 1) * CH], ob[:])
                    else:
                        nc.sync.dma_start(out_d[tok:tok + 128, n * CH:(n + 1) * CH], ob[:])
            if tail == "rs":
                emit_rs(ch)
        if tail == "rs":
            # final out DMAs last: keeps the in-order sync queue from blocking
            # mid-kernel partial writes behind collective completion waits
            for ch, rs_c in rs_outs:
                nc.sync.dma_start(out_d[ch * 128:(ch + 1) * 128, :], rs_c[:])

    nc.compile()
    return nc


_prog_cache = {}


def _get_prog(has_bv, has_bp, has_bqk):
    key = (has_bv, has_bp, has_bqk)
    if key not in _prog_cache:
        _prog_cache[key] = _build(has_bv, has_bp, has_bqk)
    return _prog_cache[key]


def _prepare(x, w_attn, b_attn, w_proj, b_proj):
    x = np.asarray(x, dtype=np.float32)
    w_attn = np.asarray(w_attn, dtype=np.float32)
    b_attn = np.asarray(b_attn, dtype=np.float32)
    w_proj = np.asarray(w_proj, dtype=np.float32)
    b_proj = np.asarray(b_proj, dtype=np.float32)

    has_bv = bool(np.any(b_attn[2 * D:]))
    has_bp = bool(np.any(b_proj))
    has_bqk = bool(np.any(b_attn[:2 * D]))
    nc = _get_prog(has_bv, has_bp, has_bqk)

    ii = np.arange(128)[:, None]
    jj = np.arange(128)[None, :]
    masks = (jj >= ii).astype(np.float32).astype(ml_dtypes.bfloat16)

    in_maps = []
    for c in range(N_CORES):
        b, g = divmod(c, 4)
        q0 = g * DG
        k0 = D + g * DG
        v0 = 2 * D + g * DG
        wv_ext = np.zeros((D, VW), dtype=np.float32)
        for hl in range(HG):
            wv_ext[:, hl * (HD + 1):hl * (HD + 1) + HD] = w_attn[:, v0 + hl * HD: v0 + (hl + 1) * HD]
        if g == 0:
            bp_tile = np.broadcast_to(b_proj, (128, D)).astype(np.float32)
        else:
            bp_tile = np.zeros((128, D), dtype=np.float32)
        in_maps.append({
            "x": np.ascontiguousarray(x[b]).astype(ml_dtypes.bfloat16),
            "wq": np.ascontiguousarray(w_attn[:, q0:q0 + DG]).astype(ml_dtypes.bfloat16),
            "wk": np.ascontiguousarray(w_attn[:, k0:k0 + DG]).astype(ml_dtypes.bfloat16),
            "wv": wv_ext.astype(ml_dtypes.bfloat16),
            "wp": np.ascontiguousarray(w_proj[g * DG:(g + 1) * DG, :]).astype(ml_dtypes.bfloat16),
            "bq": np.ascontiguousarray(b_attn[q0:q0 + DG, None]),
            "bk": np.ascontiguousarray(b_attn[k0:k0 + DG, None]),
            "bv": np.ascontiguousarray(b_attn[v0:v0 + DG, None]),
            "bp": bp_tile,
            "masks": masks,
        })
    return nc, in_maps


def _assemble(results):
    out = np.empty((B, S, D), dtype=np.float32)
    for c in range(N_CORES):
        b, g = divmod(c, 4)
        o = results[c]["out"]
        for ch in range(NCH):
            tok = ch * CH + g * 128
            out[b, tok:tok + 128, :] = o[ch * 128:(ch + 1) * 128, :]
    return out


def kernel(x, w_attn, b_attn, w_proj, b_proj):
    nc, in_maps = _prepare(x, w_attn, b_attn, w_proj, b_proj)
    res = run_bass_kernel_spmd(nc, in_maps, list(range(N_CORES)))
    return _assemble(res.results)



# revision 10
# speedup vs baseline: 1.5166x; 1.5166x over previous
"""GPT2 eager causal attention (B=2, S=2048, D=1024, H=16, HD=64) on 8 TRN2 NeuronCores.

Sharding (data + head/tensor parallel): core c -> (batch b = c//4, head-group
g = c%4), 4 heads per group.  Token ownership for the output: core (b, g) owns
token rows [ch*512 + g*128, +128) of batch b, for each 512-token chunk ch.

v2 pipeline (vs the RS baseline):
  - x is transposed on the HOST -> xT [D, S]; no transpose-DMAs on device.
  - weights are host-packed into SBUF layout -> one large DMA each, spread
    across the SP/ACT/gpsimd DMA queues so nothing serializes at startup.
  - QT/KT [256, S] and V [S, 260] (ones-column per head for the free softmax
    denominator) as in the baseline, but QKV compute for token-chunk sq>=1 is
    software-pipelined INTO the attention loop of chunk sq-1 to keep PE dense
    (HAM stays warm) and to overlap ACT-exp with PE matmuls.
  - scores: the two heads of a KT row-tile run as CONCURRENT 64-contract
    matmuls on PE row-groups 0-63/64-127 writing adjacent PSUM banks; one
    batched EXP (FD=1024) covers both.  Diagonal tiles exp the full tile
    (garbage prefix cols are simply never streamed by the AV matmul) and
    triangular-mask only the 128-wide diagonal block.
  - softmax normalize: DVE reciprocal of the ones-row directly from PSUM,
    gpsimd partition_broadcast (instead of a PE rank-1 matmul), DVE multiply.
  - c_proj: per 512-token chunk, AllToAll over the quad exchanges bf16 head
    outputs so each core receives ALL 1024 channels for ITS OWN 128-token
    slice; c_proj then contracts the full D with the full w_proj.  ~4x less
    wire than the fp32 ReduceScatter and a much shorter serial tail.
"""
from collections import deque
from contextlib import ExitStack

import ml_dtypes
import numpy as np

import concourse.bacc as bacc
import concourse.mybir as mybir
import concourse.tile as tile
from concourse.bass import ds as bass_ds
from concourse.bass_utils import run_bass_kernel_spmd

F32 = mybir.dt.float32
BF16 = mybir.dt.bfloat16
U16 = mybir.dt.uint16

B, S, D, H, HD = 2, 2048, 1024, 16, 64
N_CORES = 8
HG = 4               # heads per group
DG = HG * HD         # 256 q/k channels per group
VW = HG * (HD + 1)   # 260: 64 v-cols + 1 ones-col per head
NK = D // 128        # 8 contraction tiles over d
NS = S // 128        # 16 token tiles
CH = 512             # q-chunk (one PSUM bank of fp32)
NCH = S // CH        # 4
NRT = DG // 128      # 2 channel row-tiles (head pairs) per group
SQW = NK * CH        # 4096: xT sbuf columns per token chunk

EXP = mybir.ActivationFunctionType.Exp
IDENT = mybir.ActivationFunctionType.Identity


def _build(has_bv: bool, has_bp: bool, has_bqk: bool):
    nc = bacc.Bacc("TRN2", target_bir_lowering=False, debug=False, num_devices=N_CORES)

    xT_d = nc.dram_tensor("xT", [D, S], BF16, kind="ExternalInput").ap()
    wq_d = nc.dram_tensor("wq", [128, NK * DG], BF16, kind="ExternalInput").ap()
    wk_d = nc.dram_tensor("wk", [128, NK * DG], BF16, kind="ExternalInput").ap()
    wv_d = nc.dram_tensor("wv", [128, NK * VW], BF16, kind="ExternalInput").ap()
    wp_d = nc.dram_tensor("wp", [128, NK * D], BF16, kind="ExternalInput").ap()
    bq_d = nc.dram_tensor("bq", [DG, 1], F32, kind="ExternalInput").ap()
    bk_d = nc.dram_tensor("bk", [DG, 1], F32, kind="ExternalInput").ap()
    bv_d = nc.dram_tensor("bv", [DG, 1], F32, kind="ExternalInput").ap()
    bp_d = nc.dram_tensor("bp", [128, D], F32, kind="ExternalInput").ap()
    mk_d = nc.dram_tensor("masks", [128, 128], BF16, kind="ExternalInput").ap()
    gc_d = nc.dram_tensor("gcol", [1, 1], mybir.dt.uint32, kind="ExternalInput").ap()
    out_d = nc.dram_tensor("out", [NCH * 128, D], F32, kind="ExternalOutput").ap()

    with ExitStack() as ctx:
        tc = ctx.enter_context(tile.TileContext(nc))
        wpool = ctx.enter_context(tc.tile_pool(name="w", bufs=1))
        qkvp = ctx.enter_context(tc.tile_pool(name="qkv", bufs=1))
        stp = ctx.enter_context(tc.tile_pool(name="stx", bufs=3))
        nrm = ctx.enter_context(tc.tile_pool(name="nrm", bufs=3))
        otfp = ctx.enter_context(tc.tile_pool(name="otf", bufs=2))
        outp = ctx.enter_context(tc.tile_pool(name="outp", bufs=3))
        ps_st = ctx.enter_context(tc.tile_pool(name="psst", bufs=2, space="PSUM"))
        ps_ot = ctx.enter_context(tc.tile_pool(name="psot", bufs=2, space="PSUM"))
        ps_mm = ctx.enter_context(tc.tile_pool(name="psmm", bufs=2, space="PSUM"))
        dram = ctx.enter_context(tc.tile_pool(name="dram", bufs=1, space="DRAM"))

        # ---- tiny exp at t=0 forces the ACT table load to overlap the DMAs
        dmy = wpool.tile([128, 1], F32, tag="dmy", name="dmy")
        dmo = wpool.tile([128, 1], F32, tag="dmo", name="dmo")
        nc.vector.memset(dmy[:], 0.0)
        nc.scalar.activation(dmo[:], dmy[:], EXP, scale=1.0)

        # ---- weights: one big DMA each, spread over the ACT + gpsimd queues
        wq_sb = wpool.tile([128, NK * DG], BF16, tag="wq", name="wq_sb")
        wk_sb = wpool.tile([128, NK * DG], BF16, tag="wk", name="wk_sb")
        wv_sb = wpool.tile([128, NK * VW], BF16, tag="wv", name="wv_sb")
        wp_sb = wpool.tile([128, NK * D], BF16, tag="wp", name="wp_sb")
        mk_sb = wpool.tile([128, 128], BF16, tag="mk", name="mk_sb")
        nc.scalar.dma_start(wq_sb[:], wq_d[:])
        nc.scalar.dma_start(wk_sb[:], wk_d[:])
        nc.scalar.dma_start(wv_sb[:], wv_d[:])
        nc.scalar.dma_start(mk_sb[:], mk_d[:])
        nc.gpsimd.dma_start(wp_sb[:], wp_d[:])
        bq_sb = bk_sb = bv_sb = bp_sb = None
        if has_bqk:
            bq_sb = wpool.tile([128, NRT], F32, tag="bq", name="bq_sb")
            bk_sb = wpool.tile([128, NRT], F32, tag="bk", name="bk_sb")
            for rt in range(NRT):
                nc.scalar.dma_start(bq_sb[:, rt:rt + 1], bq_d[rt * 128:(rt + 1) * 128, :])
                nc.scalar.dma_start(bk_sb[:, rt:rt + 1], bk_d[rt * 128:(rt + 1) * 128, :])
        if has_bv:
            bv_sb = wpool.tile([128, NRT], F32, tag="bv", name="bv_sb")
            for rt in range(NRT):
                nc.scalar.dma_start(bv_sb[:, rt:rt + 1], bv_d[rt * 128:(rt + 1) * 128, :])
        if has_bp:
            bp_sb = wpool.tile([128, D], F32, tag="bp", name="bp_sb")
            nc.scalar.dma_start(bp_sb[:], bp_d[:])

        # ---- V tiles exist up-front; set the whole tile to bf16 1.0 now so
        # the later per-head copies only write the 64 v-columns
        V = []
        for st in range(NS):
            vt = qkvp.tile([128, VW], BF16, tag=f"v{st}", name=f"v{st}")
            nc.vector.memset(vt[:].bitcast(U16), 0x3F80)
            V.append(vt)

        # ---- xT loads: 4 x 1MB on the SP queue; sbuf col = sq*4096 + kt*512 + t
        xT = qkvp.tile([128, NK * S], BF16, tag="xT", name="xT")
        for sq in range(NCH):
            dst = xT[:, sq * SQW:(sq + 1) * SQW].rearrange("p (kt t) -> p kt t", kt=NK)
            src = xT_d[:, sq * CH:(sq + 1) * CH].rearrange("(kt p) t -> p kt t", kt=NK)
            nc.sync.dma_start(dst, src)

        QT = [qkvp.tile([128, S], BF16, tag=f"qt{rt}", name=f"qt{rt}") for rt in range(NRT)]
        KT = [qkvp.tile([128, S], BF16, tag=f"kt{rt}", name=f"kt{rt}") for rt in range(NRT)]
        OT = [qkvp.tile([128, S], BF16, tag=f"ot{rt}", name=f"ot{rt}") for rt in range(NRT)]

        def emit_qk(store, w_sb, b_sb, rt, sq):
            ps = ps_mm.tile([128, CH], F32, tag="ps", name=f"psqk{rt}_{sq}")
            for kt in range(NK):
                nc.tensor.matmul(
                    ps[:],
                    w_sb[:, kt * DG + rt * 128: kt * DG + (rt + 1) * 128],
                    xT[:, sq * SQW + kt * CH: sq * SQW + (kt + 1) * CH],
                    start=(kt == 0), stop=(kt == NK - 1),
                )
            dst = store[rt][:, sq * CH:(sq + 1) * CH]
            if b_sb is not None:
                nc.scalar.activation(dst, ps[:], IDENT, bias=b_sb[:, rt:rt + 1])
            else:
                nc.vector.tensor_copy(dst, ps[:])

        def emit_v(st):
            sq, j = divmod(st, 4)
            ps = ps_mm.tile([128, CH], F32, tag="ps", name=f"psv{st}")
            for kt in range(NK):
                nc.tensor.matmul(
                    ps[:, :VW],
                    xT[:, sq * SQW + kt * CH + j * 128: sq * SQW + kt * CH + (j + 1) * 128],
                    wv_sb[:, kt * VW:(kt + 1) * VW],
                    start=(kt == 0), stop=(kt == NK - 1),
                )
            for hl in range(HG):
                nc.vector.tensor_copy(
                    V[st][:, hl * (HD + 1): hl * (HD + 1) + HD],
                    ps[:, hl * (HD + 1): hl * (HD + 1) + HD],
                )

        # ---- QKV for chunk 0 up-front; chunks 1-3 become fill units that are
        # interleaved into the attention loop (PE never idles on exp waits)
        for rt in range(NRT):
            emit_qk(QT, wq_sb, bq_sb, rt, 0)
            emit_qk(KT, wk_sb, bk_sb, rt, 0)
        for st in range(4):
            emit_v(st)

        fills = {ch: deque() for ch in range(NCH)}
        for sq in range(1, NCH):
            fq = fills[sq - 1]
            fq.append(lambda sq=sq: emit_qk(QT, wq_sb, bq_sb, 0, sq))
            fq.append(lambda sq=sq: emit_qk(KT, wk_sb, bk_sb, 0, sq))
            fq.append(lambda sq=sq: emit_v(sq * 4 + 0))
            fq.append(lambda sq=sq: emit_qk(QT, wq_sb, bq_sb, 1, sq))
            fq.append(lambda sq=sq: emit_qk(KT, wk_sb, bk_sb, 1, sq))
            fq.append(lambda sq=sq: emit_v(sq * 4 + 1))
            fq.append(lambda sq=sq: emit_v(sq * 4 + 2))
            fq.append(lambda sq=sq: emit_v(sq * 4 + 3))

        # ---- collective buffers: per 512-token chunk, AllGather the quad's
        # bf16 head outputs [256 ch, 512 tok] -> [1024 ch, 512 tok]; each core
        # then reads back only ITS OWN 128-token column slice (dynamic offset
        # g*128 from the per-core "gcol" input).
        ag_in = [dram.tile([DG, CH], BF16, tag=f"agi{c}", name=f"ag_in{c}")
                 for c in range(NCH)]
        ag_out = [dram.tile([HG * DG, CH], BF16, tag=f"ago{c}", name=f"ag_out{c}")
                  for c in range(NCH)]

        gc_sb = wpool.tile([1, 1], mybir.dt.uint32, tag="gc", name="gc_sb")
        nc.sync.dma_start(gc_sb[:], gc_d[:])
        _gc_reg = nc.sync.alloc_register("gcol_reg")
        nc.sync.reg_load(_gc_reg, gc_sb[0:1, 0:1])
        gcol = nc.sync.snap(_gc_reg, donate=True, min_val=0, max_val=(HG - 1) * 128)

        def emit_ag_in(ch, rt):
            nc.sync.dma_start(
                ag_in[ch][rt * 128:(rt + 1) * 128, :],
                OT[rt][:, ch * CH:(ch + 1) * CH],
            )

        def emit_a2a(ch):
            nc.gpsimd.collective_compute(
                "AllGather",
                mybir.AluOpType.bypass,
                replica_groups=[[0, 1, 2, 3], [4, 5, 6, 7]],
                ins=[ag_in[ch].opt()],
                outs=[ag_out[ch].opt()],
            )

        otf_tiles = {}

        def emit_otf(ch):
            otf = otfp.tile([128, NK * 128], BF16, tag="otf", name=f"otf{ch}")
            src = ag_out[ch][:, bass_ds(gcol, 128)]
            nc.sync.dma_start(
                otf[:].rearrange("p (kt t) -> p kt t", kt=NK),
                src.rearrange("(kt p) t -> p kt t", kt=NK),
            )
            otf_tiles[ch] = otf

        def emit_cproj(ch):
            otf = otf_tiles[ch]
            for n in range(2):
                po = ps_mm.tile([128, CH], F32, tag="ps", name=f"po{ch}_{n}")
                for kt in range(NK):
                    nc.tensor.matmul(
                        po[:],
                        otf[:, kt * 128:(kt + 1) * 128],
                        wp_sb[:, kt * D + n * CH: kt * D + (n + 1) * CH],
                        start=(kt == 0), stop=(kt == NK - 1),
                    )
                ob = outp.tile([128, CH], F32, tag="ob", name=f"ob{ch}_{n}")
                if has_bp:
                    nc.vector.tensor_add(ob[:], po[:], bp_sb[:, n * CH:(n + 1) * CH])
                else:
                    nc.vector.tensor_copy(ob[:], po[:])
                nc.sync.dma_start(out_d[ch * 128:(ch + 1) * 128, n * CH:(n + 1) * CH], ob[:])

        def normalize(ot_ps, rt, off, ch):
            den = nrm.tile([1, CH], F32, tag="den", name=f"den{ch}_{rt}_{off}")
            nc.vector.tensor_copy(den[:], ot_ps[64:65, :])
            rden = nrm.tile([1, CH], F32, tag="rden", name=f"rden{ch}_{rt}_{off}")
            nc.vector.reciprocal_approx_fast(rden[:], den[:])
            rbc = nrm.tile([64, CH], F32, tag="rbc", name=f"rbc{ch}_{rt}_{off}")
            nc.gpsimd.partition_broadcast(rbc[:], rden[:], channels=64)
            dst = OT[rt][off:off + 64, ch * CH:(ch + 1) * CH]
            nc.vector.tensor_mul(dst, ot_ps[0:64, :], rbc[:])
            if has_bv:
                nc.vector.tensor_scalar_add(dst, dst, bv_sb[off:off + 64, rt:rt + 1])

        # ---- attention chunks, with QKV / c_proj fill units woven in.
        # cproj readback (otf) and matmuls for chunk ch-1 land in chunk ch's
        # later steps, when the AllToAll has certainly completed.
        for ch in range(NCH):
            nkt = 4 * (ch + 1)
            nsteps = 2 * nkt
            fq = fills[ch]
            n_fill = len(fq)
            emitted = 0
            step = 0
            otf_step = nkt + max(nkt // 2, 2)
            cp_step = otf_step + 2
            for rt in range(NRT):
                ot_a = ps_ot.tile([65, CH], F32, tag="ot", name=f"ota{ch}_{rt}")
                ot_b = ps_ot.tile([65, CH], F32, tag="ot", name=f"otb{ch}_{rt}")
                for kt in range(nkt):
                    st_ps = ps_st.tile([128, 2 * CH], F32, tag="st", name=f"st{ch}_{rt}_{kt}")
                    for half, off in ((0, 0), (1, 64)):
                        nc.tensor.matmul(
                            st_ps[:, half * CH:(half + 1) * CH],
                            KT[rt][off:off + 64, kt * 128:(kt + 1) * 128],
                            QT[rt][off:off + 64, ch * CH:(ch + 1) * CH],
                            start=True, stop=True,
                        )
                    st_sb = stp.tile([128, 2 * CH], BF16, tag="stsb", name=f"se{ch}_{rt}_{kt}")
                    nc.scalar.activation(st_sb[:], st_ps[:], EXP, scale=0.125)
                    d = kt - 4 * ch
                    if d >= 0:
                        for half in range(2):
                            blk = st_sb[:, half * CH + d * 128: half * CH + (d + 1) * 128]
                            nc.vector.tensor_mul(blk, blk, mk_sb[:, 0:128])
                    # fill units between the score matmuls and the AV matmuls
                    while emitted * nsteps < (step + 1) * n_fill:
                        fq.popleft()()
                        emitted += 1
                    if ch > 0:
                        if step == otf_step:
                            emit_otf(ch - 1)
                        elif step == cp_step:
                            emit_cproj(ch - 1)
                    lo = max(d, 0) * 128 if d > 0 else 0
                    for half, ot_ps in ((0, ot_a), (1, ot_b)):
                        nc.tensor.matmul(
                            ot_ps[:, lo:],
                            V[kt][:, (rt * 2 + half) * (HD + 1):(rt * 2 + half + 1) * (HD + 1)],
                            st_sb[:, half * CH + lo:(half + 1) * CH],
                            start=(kt == 0), stop=(kt == nkt - 1),
                        )
                    step += 1
                normalize(ot_a, rt, 0, ch)
                normalize(ot_b, rt, 64, ch)
                emit_ag_in(ch, rt)
            emit_a2a(ch)
        emit_otf(NCH - 1)
        emit_cproj(NCH - 1)

    nc.compile()
    return nc


_prog_cache = {}


def _get_prog(has_bv, has_bp, has_bqk):
    key = (has_bv, has_bp, has_bqk)
    if key not in _prog_cache:
        _prog_cache[key] = _build(*key)
    return _prog_cache[key]


def _pack_kmajor(w):
    # [D, X] -> [128, NK*X] with col = kt*X + x, so sbuf[:, kt*X+x] = w[kt*128+p, x]
    dd, x = w.shape
    return np.ascontiguousarray(
        w.reshape(NK, 128, x).transpose(1, 0, 2).reshape(128, NK * x)
    )


def _prepare(x, w_attn, b_attn, w_proj, b_proj):
    x = np.asarray(x, dtype=np.float32)
    w_attn = np.asarray(w_attn, dtype=np.float32)
    b_attn = np.asarray(b_attn, dtype=np.float32)
    w_proj = np.asarray(w_proj, dtype=np.float32)
    b_proj = np.asarray(b_proj, dtype=np.float32)

    has_bv = bool(np.any(b_attn[2 * D:]))
    has_bp = bool(np.any(b_proj))
    has_bqk = bool(np.any(b_attn[:2 * D]))
    nc = _get_prog(has_bv, has_bp, has_bqk)

    ii = np.arange(128)[:, None]
    jj = np.arange(128)[None, :]
    masks = (jj >= ii).astype(np.float32).astype(ml_dtypes.bfloat16)

    # per-batch transposed activations (shared across the 4 group-cores)
    xT_b = [np.ascontiguousarray(x[b].T).astype(ml_dtypes.bfloat16) for b in range(B)]
    # per-group packed weights (shared across the 2 batch-cores)
    wq_g, wk_g, wv_g, bq_g, bk_g, bv_g = [], [], [], [], [], []
    for g in range(HG):
        q0 = g * DG
        k0 = D + g * DG
        v0 = 2 * D + g * DG
        wv_ext = np.zeros((D, VW), dtype=np.float32)
        for hl in range(HG):
            wv_ext[:, hl * (HD + 1):hl * (HD + 1) + HD] = \
                w_attn[:, v0 + hl * HD: v0 + (hl + 1) * HD]
        wq_g.append(_pack_kmajor(w_attn[:, q0:q0 + DG]).astype(ml_dtypes.bfloat16))
        wk_g.append(_pack_kmajor(w_attn[:, k0:k0 + DG]).astype(ml_dtypes.bfloat16))
        wv_g.append(_pack_kmajor(wv_ext).astype(ml_dtypes.bfloat16))
        bq_g.append(np.ascontiguousarray(b_attn[q0:q0 + DG, None]))
        bk_g.append(np.ascontiguousarray(b_attn[k0:k0 + DG, None]))
        bv_g.append(np.ascontiguousarray(b_attn[v0:v0 + DG, None]))
    wp_pack = _pack_kmajor(w_proj).astype(ml_dtypes.bfloat16)
    bp_tile = np.broadcast_to(b_proj, (128, D)).astype(np.float32)

    in_maps = []
    for c in range(N_CORES):
        b, g = divmod(c, 4)
        in_maps.append({
            "xT": xT_b[b],
            "wq": wq_g[g],
            "wk": wk_g[g],
            "wv": wv_g[g],
            "wp": wp_pack,
            "bq": bq_g[g],
            "bk": bk_g[g],
            "bv": bv_g[g],
            "bp": bp_tile,
            "masks": masks,
            "gcol": np.array([[g * 128]], dtype=np.uint32),
        })
    return nc, in_maps


def _assemble(results):
    out = np.empty((B, S, D), dtype=np.float32)
    for c in range(N_CORES):
        b, g = divmod(c, 4)
        o = results[c]["out"]
        for ch in range(NCH):
            tok = ch * CH + g * 128
            out[b, tok:tok + 128, :] = o[ch * 128:(ch + 1) * 128, :]
    return out


def kernel(x, w_attn, b_attn, w_proj, b_proj):
    nc, in_maps = _prepare(x, w_attn, b_attn, w_proj, b_proj)
    res = run_bass_kernel_spmd(nc, in_maps, list(range(N_CORES)))
    return _assemble(res.results)


# revision 15
# speedup vs baseline: 1.5641x; 1.0313x over previous
"""GPT2 eager causal attention (B=2, S=2048, D=1024, H=16, HD=64) on 8 TRN2 NeuronCores.

Sharding (data + head/tensor parallel): core c -> (batch b = c//4, head-group
g = c%4), 4 heads per group.  Token ownership for the output: core (b, g) owns
token rows [ch*512 + g*128, +128) of batch b, for each 512-token chunk ch.

v2 pipeline (vs the RS baseline):
  - x is transposed on the HOST -> xT [D, S]; no transpose-DMAs on device.
  - weights are host-packed into SBUF layout -> one large DMA each, spread
    across the SP/ACT/gpsimd DMA queues so nothing serializes at startup.
  - QT/KT [256, S] and V [S, 260] (ones-column per head for the free softmax
    denominator) as in the baseline, but QKV compute for token-chunk sq>=1 is
    software-pipelined INTO the attention loop of chunk sq-1 to keep PE dense
    (HAM stays warm) and to overlap ACT-exp with PE matmuls.
  - scores: the two heads of a KT row-tile run as CONCURRENT 64-contract
    matmuls on PE row-groups 0-63/64-127 writing adjacent PSUM banks; one
    batched EXP (FD=1024) covers both.  Diagonal tiles exp the full tile
    (garbage prefix cols are simply never streamed by the AV matmul) and
    triangular-mask only the 128-wide diagonal block.
  - softmax normalize: DVE reciprocal of the ones-row directly from PSUM,
    gpsimd partition_broadcast (instead of a PE rank-1 matmul), DVE multiply.
  - c_proj: per 512-token chunk, AllToAll over the quad exchanges bf16 head
    outputs so each core receives ALL 1024 channels for ITS OWN 128-token
    slice; c_proj then contracts the full D with the full w_proj.  ~4x less
    wire than the fp32 ReduceScatter and a much shorter serial tail.
"""
from collections import deque
from contextlib import ExitStack

import ml_dtypes
import numpy as np

import concourse.bacc as bacc
import concourse.mybir as mybir
import concourse.tile as tile
from concourse.bass import ds as bass_ds
from concourse.bass_utils import run_bass_kernel_spmd

F32 = mybir.dt.float32
BF16 = mybir.dt.bfloat16
U16 = mybir.dt.uint16

B, S, D, H, HD = 2, 2048, 1024, 16, 64
N_CORES = 8
HG = 4               # heads per group
DG = HG * HD         # 256 q/k channels per group
VW = HG * (HD + 1)   # 260: 64 v-cols + 1 ones-col per head
NK = D // 128        # 8 contraction tiles over d
NS = S // 128        # 16 token tiles
CH = 512             # q-chunk (one PSUM bank of fp32)
NCH = S // CH        # 4
NRT = DG // 128      # 2 channel row-tiles (head pairs) per group
SQW = NK * CH        # 4096: xT sbuf columns per token chunk

EXP = mybir.ActivationFunctionType.Exp
IDENT = mybir.ActivationFunctionType.Identity


def _build(has_bv: bool, has_bp: bool, has_bqk: bool):
    nc = bacc.Bacc("TRN2", target_bir_lowering=False, debug=False, num_devices=N_CORES)

    xT_d = nc.dram_tensor("xT", [128, NK * S], BF16, kind="ExternalInput").ap()
    wq_d = nc.dram_tensor("wq", [128, NK * DG], BF16, kind="ExternalInput").ap()
    wk_d = nc.dram_tensor("wk", [128, NK * DG], BF16, kind="ExternalInput").ap()
    wv_d = nc.dram_tensor("wv", [128, NK * VW], BF16, kind="ExternalInput").ap()
    wp_d = nc.dram_tensor("wp", [128, NK * D], BF16, kind="ExternalInput").ap()
    bq_d = nc.dram_tensor("bq", [DG, 1], F32, kind="ExternalInput").ap()
    bk_d = nc.dram_tensor("bk", [DG, 1], F32, kind="ExternalInput").ap()
    bv_d = nc.dram_tensor("bv", [DG, 1], F32, kind="ExternalInput").ap()
    bp_d = nc.dram_tensor("bp", [128, D], F32, kind="ExternalInput").ap()
    mk_d = nc.dram_tensor("masks", [128, 128], BF16, kind="ExternalInput").ap()
    gc_d = nc.dram_tensor("gcol", [1, 1], mybir.dt.uint32, kind="ExternalInput").ap()
    out_d = nc.dram_tensor("out", [NCH * 128, D], F32, kind="ExternalOutput").ap()

    with ExitStack() as ctx:
        tc = ctx.enter_context(tile.TileContext(nc))
        wpool = ctx.enter_context(tc.tile_pool(name="w", bufs=1))
        qkvp = ctx.enter_context(tc.tile_pool(name="qkv", bufs=1))
        stp = ctx.enter_context(tc.tile_pool(name="stx", bufs=3))
        nrm = ctx.enter_context(tc.tile_pool(name="nrm", bufs=3))
        otfp = ctx.enter_context(tc.tile_pool(name="otf", bufs=2))
        outp = ctx.enter_context(tc.tile_pool(name="outp", bufs=3))
        ps_st = ctx.enter_context(tc.tile_pool(name="psst", bufs=2, space="PSUM"))
        ps_ot = ctx.enter_context(tc.tile_pool(name="psot", bufs=2, space="PSUM"))
        ps_mm = ctx.enter_context(tc.tile_pool(name="psmm", bufs=2, space="PSUM"))
        dram = ctx.enter_context(tc.tile_pool(name="dram", bufs=1, space="DRAM"))

        # ---- tiny exp at t=0 forces the ACT table load to overlap the DMAs
        dmy = wpool.tile([128, 1], F32, tag="dmy", name="dmy")
        dmo = wpool.tile([128, 1], F32, tag="dmo", name="dmo")
        nc.vector.memset(dmy[:], 0.0)
        nc.scalar.activation(dmo[:], dmy[:], EXP, scale=1.0)

        # ---- weights: one big DMA each, spread over the ACT + gpsimd queues
        wq_sb = wpool.tile([128, NK * DG], BF16, tag="wq", name="wq_sb")
        wk_sb = wpool.tile([128, NK * DG], BF16, tag="wk", name="wk_sb")
        wv_sb = wpool.tile([128, NK * VW], BF16, tag="wv", name="wv_sb")
        wp_sb = wpool.tile([128, NK * D], BF16, tag="wp", name="wp_sb")
        mk_sb = wpool.tile([128, 128], BF16, tag="mk", name="mk_sb")
        nc.scalar.dma_start(wq_sb[:], wq_d[:])
        nc.scalar.dma_start(wk_sb[:], wk_d[:])
        nc.scalar.dma_start(wv_sb[:], wv_d[:])
        nc.scalar.dma_start(mk_sb[:], mk_d[:])
        nc.gpsimd.dma_start(wp_sb[:], wp_d[:])
        bq_sb = bk_sb = bv_sb = bp_sb = None
        if has_bqk:
            bq_sb = wpool.tile([128, NRT], F32, tag="bq", name="bq_sb")
            bk_sb = wpool.tile([128, NRT], F32, tag="bk", name="bk_sb")
            for rt in range(NRT):
                nc.scalar.dma_start(bq_sb[:, rt:rt + 1], bq_d[rt * 128:(rt + 1) * 128, :])
                nc.scalar.dma_start(bk_sb[:, rt:rt + 1], bk_d[rt * 128:(rt + 1) * 128, :])
        if has_bv:
            bv_sb = wpool.tile([128, NRT], F32, tag="bv", name="bv_sb")
            for rt in range(NRT):
                nc.scalar.dma_start(bv_sb[:, rt:rt + 1], bv_d[rt * 128:(rt + 1) * 128, :])
        if has_bp:
            bp_sb = wpool.tile([128, D], F32, tag="bp", name="bp_sb")
            nc.scalar.dma_start(bp_sb[:], bp_d[:])

        # ---- V tiles exist up-front; set the whole tile to bf16 1.0 now so
        # the later per-head copies only write the 64 v-columns
        V = []
        for st in range(NS):
            vt = qkvp.tile([128, VW], BF16, tag=f"v{st}", name=f"v{st}")
            nc.vector.memset(vt[:].bitcast(U16), 0x3F80)
            V.append(vt)

        # ---- xT loads: host pre-packed to SBUF layout (col = sq*4096 + kt*512
        # + t), so each chunk is one fully-contiguous 1MB DMA
        xT = qkvp.tile([128, NK * S], BF16, tag="xT", name="xT")
        for sq in range(NCH):
            nc.sync.dma_start(
                xT[:, sq * SQW:(sq + 1) * SQW], xT_d[:, sq * SQW:(sq + 1) * SQW]
            )

        QT = [qkvp.tile([128, S], BF16, tag=f"qt{rt}", name=f"qt{rt}") for rt in range(NRT)]
        KT = [qkvp.tile([128, S], BF16, tag=f"kt{rt}", name=f"kt{rt}") for rt in range(NRT)]
        OT = [qkvp.tile([128, S], BF16, tag=f"ot{rt}", name=f"ot{rt}") for rt in range(NRT)]

        def emit_qk(store, w_sb, b_sb, rt, sq):
            ps = ps_mm.tile([128, CH], F32, tag="ps", name=f"psqk{rt}_{sq}")
            for kt in range(NK):
                nc.tensor.matmul(
                    ps[:],
                    w_sb[:, kt * DG + rt * 128: kt * DG + (rt + 1) * 128],
                    xT[:, sq * SQW + kt * CH: sq * SQW + (kt + 1) * CH],
                    start=(kt == 0), stop=(kt == NK - 1),
                )
            dst = store[rt][:, sq * CH:(sq + 1) * CH]
            if b_sb is not None:
                nc.scalar.activation(dst, ps[:], IDENT, bias=b_sb[:, rt:rt + 1])
            else:
                nc.vector.tensor_copy(dst, ps[:])

        def emit_v(st):
            sq, j = divmod(st, 4)
            ps = ps_mm.tile([128, CH], F32, tag="ps", name=f"psv{st}")
            for kt in range(NK):
                nc.tensor.matmul(
                    ps[:, :VW],
                    xT[:, sq * SQW + kt * CH + j * 128: sq * SQW + kt * CH + (j + 1) * 128],
                    wv_sb[:, kt * VW:(kt + 1) * VW],
                    start=(kt == 0), stop=(kt == NK - 1),
                )
            for hl in range(HG):
                nc.vector.tensor_copy(
                    V[st][:, hl * (HD + 1): hl * (HD + 1) + HD],
                    ps[:, hl * (HD + 1): hl * (HD + 1) + HD],
                )

        # ---- QKV for chunk 0 up-front; chunks 1-3 become fill units that are
        # interleaved into the attention loop (PE never idles on exp waits)
        for rt in range(NRT):
            emit_qk(QT, wq_sb, bq_sb, rt, 0)
            emit_qk(KT, wk_sb, bk_sb, rt, 0)
        for st in range(4):
            emit_v(st)

        fills = {ch: deque() for ch in range(NCH)}
        for sq in range(1, NCH):
            fq = fills[sq - 1]
            fq.append(lambda sq=sq: emit_qk(QT, wq_sb, bq_sb, 0, sq))
            fq.append(lambda sq=sq: emit_qk(KT, wk_sb, bk_sb, 0, sq))
            fq.append(lambda sq=sq: emit_v(sq * 4 + 0))
            fq.append(lambda sq=sq: emit_qk(QT, wq_sb, bq_sb, 1, sq))
            fq.append(lambda sq=sq: emit_qk(KT, wk_sb, bk_sb, 1, sq))
            fq.append(lambda sq=sq: emit_v(sq * 4 + 1))
            fq.append(lambda sq=sq: emit_v(sq * 4 + 2))
            fq.append(lambda sq=sq: emit_v(sq * 4 + 3))

        # ---- collective buffers: per 512-token chunk, AllGather the quad's
        # bf16 head outputs [256 ch, 512 tok] -> [1024 ch, 512 tok]; each core
        # then reads back only ITS OWN 128-token column slice (dynamic offset
        # g*128 from the per-core "gcol" input).
        ag_in = [dram.tile([DG, CH], BF16, tag=f"agi{c}", name=f"ag_in{c}")
                 for c in range(NCH)]
        ag_out = [dram.tile([HG * DG, CH], BF16, tag=f"ago{c}", name=f"ag_out{c}")
                  for c in range(NCH)]

        gc_sb = wpool.tile([1, 1], mybir.dt.uint32, tag="gc", name="gc_sb")
        nc.sync.dma_start(gc_sb[:], gc_d[:])
        _gc_reg = nc.sync.alloc_register("gcol_reg")
        nc.sync.reg_load(_gc_reg, gc_sb[0:1, 0:1])
        gcol = nc.sync.snap(_gc_reg, donate=True, min_val=0, max_val=(HG - 1) * 128)

        # ---- tiny warm-up AllGather at t~0 absorbs the ncfw cold-start
        # (~15-25us) while the weight DMAs stream in
        wa_sb = wpool.tile([1, 16], F32, tag="wa", name="wa_sb")
        nc.vector.memset(wa_sb[:], 0.0)
        wa_in = dram.tile([1, 16], F32, tag="wain", name="wa_in")
        wa_out = dram.tile([4, 16], F32, tag="waout", name="wa_out")
        nc.sync.dma_start(wa_in[:], wa_sb[:])
        nc.gpsimd.collective_compute(
            "AllGather",
            mybir.AluOpType.bypass,
            replica_groups=[[0, 1, 2, 3], [4, 5, 6, 7]],
            ins=[wa_in.opt()],
            outs=[wa_out.opt()],
        )

        def emit_ag_in(ch, rt):
            nc.sync.dma_start(
                ag_in[ch][rt * 128:(rt + 1) * 128, :],
                OT[rt][:, ch * CH:(ch + 1) * CH],
            )

        def emit_a2a(ch):
            nc.gpsimd.collective_compute(
                "AllGather",
                mybir.AluOpType.bypass,
                replica_groups=[[0, 1, 2, 3], [4, 5, 6, 7]],
                ins=[ag_in[ch].opt()],
                outs=[ag_out[ch].opt()],
            )

        otf_tiles = {}

        def emit_otf(ch):
            otf = otfp.tile([128, NK * 128], BF16, tag="otf", name=f"otf{ch}")
            src = ag_out[ch][:, bass_ds(gcol, 128)]
            nc.sync.dma_start(
                otf[:].rearrange("p (kt t) -> p kt t", kt=NK),
                src.rearrange("(kt p) t -> p kt t", kt=NK),
            )
            otf_tiles[ch] = otf

        def emit_cproj(ch):
            otf = otf_tiles[ch]
            for n in range(2):
                po = ps_mm.tile([128, CH], F32, tag="ps", name=f"po{ch}_{n}")
                for kt in range(NK):
                    nc.tensor.matmul(
                        po[:],
                        otf[:, kt * 128:(kt + 1) * 128],
                        wp_sb[:, kt * D + n * CH: kt * D + (n + 1) * CH],
                        start=(kt == 0), stop=(kt == NK - 1),
                    )
                ob = outp.tile([128, CH], F32, tag="ob", name=f"ob{ch}_{n}")
                if has_bp:
                    nc.vector.tensor_add(ob[:], po[:], bp_sb[:, n * CH:(n + 1) * CH])
                else:
                    nc.vector.tensor_copy(ob[:], po[:])
                nc.sync.dma_start(out_d[ch * 128:(ch + 1) * 128, n * CH:(n + 1) * CH], ob[:])

        def normalize(ot_ps, rt, off, ch):
            den = nrm.tile([1, CH], F32, tag="den", name=f"den{ch}_{rt}_{off}")
            nc.vector.tensor_copy(den[:], ot_ps[64:65, :])
            rden = nrm.tile([1, CH], F32, tag="rden", name=f"rden{ch}_{rt}_{off}")
            nc.vector.reciprocal_approx_fast(rden[:], den[:])
            rbc = nrm.tile([64, CH], F32, tag="rbc", name=f"rbc{ch}_{rt}_{off}")
            nc.gpsimd.partition_broadcast(rbc[:], rden[:], channels=64)
            dst = OT[rt][off:off + 64, ch * CH:(ch + 1) * CH]
            nc.vector.tensor_mul(dst, ot_ps[0:64, :], rbc[:])
            if has_bv:
                nc.vector.tensor_scalar_add(dst, dst, bv_sb[off:off + 64, rt:rt + 1])

        # ---- attention chunks, with QKV fill units woven in.  cproj for
        # chunk ch-1 (otf readback + matmuls) is emitted right after chunk
        # ch's AllGather trigger: by then AG(ch-1) has completed, so neither
        # the PE FIFO nor the SP DMA queue ever blocks on a collective
        # mid-stream — only the final chunk's AG is exposed.
        for ch in range(NCH):
            nkt = 4 * (ch + 1)
            nsteps = 2 * nkt
            fq = fills[ch]
            n_fill = len(fq)
            emitted = 0
            step = 0
            for rt in range(NRT):
                ot_a = ps_ot.tile([65, CH], F32, tag="ot", name=f"ota{ch}_{rt}")
                ot_b = ps_ot.tile([65, CH], F32, tag="ot", name=f"otb{ch}_{rt}")
                for kt in range(nkt):
                    st_ps = ps_st.tile([128, 2 * CH], F32, tag="st", name=f"st{ch}_{rt}_{kt}")
                    for half, off in ((0, 0), (1, 64)):
                        nc.tensor.matmul(
                            st_ps[:, half * CH:(half + 1) * CH],
                            KT[rt][off:off + 64, kt * 128:(kt + 1) * 128],
                            QT[rt][off:off + 64, ch * CH:(ch + 1) * CH],
                            start=True, stop=True,
                        )
                    st_sb = stp.tile([128, 2 * CH], BF16, tag="stsb", name=f"se{ch}_{rt}_{kt}")
                    nc.scalar.activation(st_sb[:], st_ps[:], EXP, scale=0.125)
                    d = kt - 4 * ch
                    if d >= 0:
                        for half in range(2):
                            blk = st_sb[:, half * CH + d * 128: half * CH + (d + 1) * 128]
                            nc.vector.tensor_mul(blk, blk, mk_sb[:, 0:128])
                    # fill units between the score matmuls and the AV matmuls
                    while emitted * nsteps < (step + 1) * n_fill:
                        fq.popleft()()
                        emitted += 1
                    lo = max(d, 0) * 128 if d > 0 else 0
                    for half, ot_ps in ((0, ot_a), (1, ot_b)):
                        nc.tensor.matmul(
                            ot_ps[:, lo:],
                            V[kt][:, (rt * 2 + half) * (HD + 1):(rt * 2 + half + 1) * (HD + 1)],
                            st_sb[:, half * CH + lo:(half + 1) * CH],
                            start=(kt == 0), stop=(kt == nkt - 1),
                        )
                    step += 1
                normalize(ot_a, rt, 0, ch)
                normalize(ot_b, rt, 64, ch)
                emit_ag_in(ch, rt)
            emit_a2a(ch)
            if ch > 0:
                emit_otf(ch - 1)
                emit_cproj(ch - 1)
        emit_otf(NCH - 1)
        emit_cproj(NCH - 1)

    nc.compile()
    return nc


_prog_cache = {}


def _get_prog(has_bv, has_bp, has_bqk):
    key = (has_bv, has_bp, has_bqk)
    if key not in _prog_cache:
        _prog_cache[key] = _build(*key)
    return _prog_cache[key]


def _pack_kmajor(w):
    # [D, X] -> [128, NK*X] with col = kt*X + x, so sbuf[:, kt*X+x] = w[kt*128+p, x]
    dd, x = w.shape
    return np.ascontiguousarray(
        w.reshape(NK, 128, x).transpose(1, 0, 2).reshape(128, NK * x)
    )


def _prepare(x, w_attn, b_attn, w_proj, b_proj):
    x = np.asarray(x, dtype=np.float32)
    w_attn = np.asarray(w_attn, dtype=np.float32)
    b_attn = np.asarray(b_attn, dtype=np.float32)
    w_proj = np.asarray(w_proj, dtype=np.float32)
    b_proj = np.asarray(b_proj, dtype=np.float32)

    has_bv = bool(np.any(b_attn[2 * D:]))
    has_bp = bool(np.any(b_proj))
    has_bqk = bool(np.any(b_attn[:2 * D]))
    nc = _get_prog(has_bv, has_bp, has_bqk)

    ii = np.arange(128)[:, None]
    jj = np.arange(128)[None, :]
    masks = (jj >= ii).astype(np.float32).astype(ml_dtypes.bfloat16)

    # per-batch activations, transposed + packed to the SBUF layout
    # (col = sq*4096 + kt*512 + t), shared across the 4 group-cores
    xT_b = [
        np.ascontiguousarray(
            x[b].reshape(NCH, CH, NK, 128).transpose(3, 0, 2, 1).reshape(128, NK * S)
        ).astype(ml_dtypes.bfloat16)
        for b in range(B)
    ]
    # per-group packed weights (shared across the 2 batch-cores)
    wq_g, wk_g, wv_g, bq_g, bk_g, bv_g = [], [], [], [], [], []
    for g in range(HG):
        q0 = g * DG
        k0 = D + g * DG
        v0 = 2 * D + g * DG
        wv_ext = np.zeros((D, VW), dtype=np.float32)
        for hl in range(HG):
            wv_ext[:, hl * (HD + 1):hl * (HD + 1) + HD] = \
                w_attn[:, v0 + hl * HD: v0 + (hl + 1) * HD]
        wq_g.append(_pack_kmajor(w_attn[:, q0:q0 + DG]).astype(ml_dtypes.bfloat16))
        wk_g.append(_pack_kmajor(w_attn[:, k0:k0 + DG]).astype(ml_dtypes.bfloat16))
        wv_g.append(_pack_kmajor(wv_ext).astype(ml_dtypes.bfloat16))
        bq_g.append(np.ascontiguousarray(b_attn[q0:q0 + DG, None]))
        bk_g.append(np.ascontiguousarray(b_attn[k0:k0 + DG, None]))
        bv_g.append(np.ascontiguousarray(b_attn[v0:v0 + DG, None]))
    wp_pack = _pack_kmajor(w_proj).astype(ml_dtypes.bfloat16)
    bp_tile = np.broadcast_to(b_proj, (128, D)).astype(np.float32)

    in_maps = []
    for c in range(N_CORES):
        b, g = divmod(c, 4)
        in_maps.append({
            "xT": xT_b[b],
            "wq": wq_g[g],
            "wk": wk_g[g],
            "wv": wv_g[g],
            "wp": wp_pack,
            "bq": bq_g[g],
            "bk": bk_g[g],
            "bv": bv_g[g],
            "bp": bp_tile,
            "masks": masks,
            "gcol": np.array([[g * 128]], dtype=np.uint32),
        })
    return nc, in_maps


def _assemble(results):
    out = np.empty((B, S, D), dtype=np.float32)
    for c in range(N_CORES):
        b, g = divmod(c, 4)
        o = results[c]["out"]
        for ch in range(NCH):
            tok = ch * CH + g * 128
            out[b, tok:tok + 128, :] = o[ch * 128:(ch + 1) * 128, :]
    return out


def kernel(x, w_attn, b_attn, w_proj, b_proj):
    nc, in_maps = _prepare(x, w_attn, b_attn, w_proj, b_proj)
    res = run_bass_kernel_spmd(nc, in_maps, list(range(N_CORES)))
    return _assemble(res.results)


# revision 21
# speedup vs baseline: 1.6509x; 1.0555x over previous
"""GPT2 eager causal attention (B=2, S=2048, D=1024, H=16, HD=64) on 8 TRN2 NeuronCores.

Sharding (data + head/tensor parallel): core c -> (batch b = c//4, head-group
g = c%4), 4 heads per group.  Token ownership for the output: core (b, g) owns
token rows [ch*512 + g*128, +128) of batch b, for each 512-token chunk ch.

v2 pipeline (vs the RS baseline):
  - x is transposed on the HOST -> xT [D, S]; no transpose-DMAs on device.
  - weights are host-packed into SBUF layout -> one large DMA each, spread
    across the SP/ACT/gpsimd DMA queues so nothing serializes at startup.
  - QT/KT [256, S] and V [S, 260] (ones-column per head for the free softmax
    denominator) as in the baseline, but QKV compute for token-chunk sq>=1 is
    software-pipelined INTO the attention loop of chunk sq-1 to keep PE dense
    (HAM stays warm) and to overlap ACT-exp with PE matmuls.
  - scores: the two heads of a KT row-tile run as CONCURRENT 64-contract
    matmuls on PE row-groups 0-63/64-127 writing adjacent PSUM banks; one
    batched EXP (FD=1024) covers both.  Diagonal tiles exp the full tile
    (garbage prefix cols are simply never streamed by the AV matmul) and
    triangular-mask only the 128-wide diagonal block.
  - softmax normalize: DVE reciprocal of the ones-row directly from PSUM,
    gpsimd partition_broadcast (instead of a PE rank-1 matmul), DVE multiply.
  - c_proj: per 512-token chunk, AllToAll over the quad exchanges bf16 head
    outputs so each core receives ALL 1024 channels for ITS OWN 128-token
    slice; c_proj then contracts the full D with the full w_proj.  ~4x less
    wire than the fp32 ReduceScatter and a much shorter serial tail.
"""
from collections import deque
from contextlib import ExitStack

import ml_dtypes
import numpy as np

import concourse.bacc as bacc
import concourse.mybir as mybir
import concourse.tile as tile
from concourse.bass import ds as bass_ds
from concourse.bass_utils import run_bass_kernel_spmd

F32 = mybir.dt.float32
BF16 = mybir.dt.bfloat16
U16 = mybir.dt.uint16

B, S, D, H, HD = 2, 2048, 1024, 16, 64
N_CORES = 8
HG = 4               # heads per group
DG = HG * HD         # 256 q/k channels per group
VW = HG * (HD + 1)   # 260: 64 v-cols + 1 ones-col per head
NK = D // 128        # 8 contraction tiles over d
NS = S // 128        # 16 token tiles
CH = 512             # q-chunk (one PSUM bank of fp32)
NCH = S // CH        # 4
NRT = DG // 128      # 2 channel row-tiles (head pairs) per group
SQW = NK * CH        # 4096: xT sbuf columns per token chunk

EXP = mybir.ActivationFunctionType.Exp
IDENT = mybir.ActivationFunctionType.Identity


def _build(has_bv: bool, has_bp: bool, has_bqk: bool):
    nc = bacc.Bacc("TRN2", target_bir_lowering=False, debug=False, num_devices=N_CORES)

    xT_d = nc.dram_tensor("xT", [128, NK * S], BF16, kind="ExternalInput").ap()
    wq_d = nc.dram_tensor("wq", [128, NK * DG], BF16, kind="ExternalInput").ap()
    wk_d = nc.dram_tensor("wk", [128, NK * DG], BF16, kind="ExternalInput").ap()
    wv_d = nc.dram_tensor("wv", [128, NK * VW], BF16, kind="ExternalInput").ap()
    wp_d = nc.dram_tensor("wp", [128, NK * D], BF16, kind="ExternalInput").ap()
    bq_d = nc.dram_tensor("bq", [DG, 1], F32, kind="ExternalInput").ap()
    bk_d = nc.dram_tensor("bk", [DG, 1], F32, kind="ExternalInput").ap()
    bv_d = nc.dram_tensor("bv", [DG, 1], F32, kind="ExternalInput").ap()
    bp_d = nc.dram_tensor("bp", [128, D], F32, kind="ExternalInput").ap()
    mk_d = nc.dram_tensor("masks", [128, 128], BF16, kind="ExternalInput").ap()
    gc_d = nc.dram_tensor("gcol", [1, 1], mybir.dt.uint32, kind="ExternalInput").ap()
    out_d = nc.dram_tensor("out", [NCH * 128, D], F32, kind="ExternalOutput").ap()

    with ExitStack() as ctx:
        tc = ctx.enter_context(tile.TileContext(nc))
        wpool = ctx.enter_context(tc.tile_pool(name="w", bufs=1))
        qkvp = ctx.enter_context(tc.tile_pool(name="qkv", bufs=1))
        stp = ctx.enter_context(tc.tile_pool(name="stx", bufs=3))
        nrm = ctx.enter_context(tc.tile_pool(name="nrm", bufs=3))
        otfp = ctx.enter_context(tc.tile_pool(name="otf", bufs=2))
        outp = ctx.enter_context(tc.tile_pool(name="outp", bufs=3))
        ps_st = ctx.enter_context(tc.tile_pool(name="psst", bufs=2, space="PSUM"))
        ps_ot = ctx.enter_context(tc.tile_pool(name="psot", bufs=2, space="PSUM"))
        ps_mm = ctx.enter_context(tc.tile_pool(name="psmm", bufs=2, space="PSUM"))
        dram = ctx.enter_context(tc.tile_pool(name="dram", bufs=1, space="DRAM"))

        # ---- tiny exp at t=0 forces the ACT table load to overlap the DMAs
        dmy = wpool.tile([128, 1], F32, tag="dmy", name="dmy")
        dmo = wpool.tile([128, 1], F32, tag="dmo", name="dmo")
        nc.vector.memset(dmy[:], 0.0)
        nc.scalar.activation(dmo[:], dmy[:], EXP, scale=1.0)

        # ---- weights: one big DMA each, spread over the ACT + gpsimd queues
        wq_sb = wpool.tile([128, NK * DG], BF16, tag="wq", name="wq_sb")
        wk_sb = wpool.tile([128, NK * DG], BF16, tag="wk", name="wk_sb")
        wv_sb = wpool.tile([128, NK * VW], BF16, tag="wv", name="wv_sb")
        wp_sb = wpool.tile([128, NK * D], BF16, tag="wp", name="wp_sb")
        mk_sb = wpool.tile([128, 128], BF16, tag="mk", name="mk_sb")
        nc.scalar.dma_start(wq_sb[:], wq_d[:])
        nc.scalar.dma_start(wk_sb[:], wk_d[:])
        nc.scalar.dma_start(mk_sb[:], mk_d[:])
        nc.gpsimd.dma_start(wv_sb[:], wv_d[:])
        nc.gpsimd.dma_start(wp_sb[:], wp_d[:])
        bq_sb = bk_sb = bv_sb = bp_sb = None
        if has_bqk:
            bq_sb = wpool.tile([128, NRT], F32, tag="bq", name="bq_sb")
            bk_sb = wpool.tile([128, NRT], F32, tag="bk", name="bk_sb")
            for rt in range(NRT):
                nc.scalar.dma_start(bq_sb[:, rt:rt + 1], bq_d[rt * 128:(rt + 1) * 128, :])
                nc.scalar.dma_start(bk_sb[:, rt:rt + 1], bk_d[rt * 128:(rt + 1) * 128, :])
        if has_bv:
            bv_sb = wpool.tile([128, NRT], F32, tag="bv", name="bv_sb")
            for rt in range(NRT):
                nc.scalar.dma_start(bv_sb[:, rt:rt + 1], bv_d[rt * 128:(rt + 1) * 128, :])
        if has_bp:
            bp_sb = wpool.tile([128, D], F32, tag="bp", name="bp_sb")
            nc.scalar.dma_start(bp_sb[:], bp_d[:])

        # ---- V tiles exist up-front; set the whole tile to bf16 1.0 now so
        # the later per-head copies only write the 64 v-columns
        V = []
        for st in range(NS):
            vt = qkvp.tile([128, VW], BF16, tag=f"v{st}", name=f"v{st}")
            nc.vector.memset(vt[:].bitcast(U16), 0x3F80)
            V.append(vt)

        # ---- xT loads: host pre-packed to SBUF layout (col = sq*4096 + kt*512
        # + t), so each chunk is one fully-contiguous 1MB DMA
        xT = qkvp.tile([128, NK * S], BF16, tag="xT", name="xT")
        for sq in range(NCH):
            nc.sync.dma_start(
                xT[:, sq * SQW:(sq + 1) * SQW], xT_d[:, sq * SQW:(sq + 1) * SQW]
            )

        QT = [qkvp.tile([128, S], BF16, tag=f"qt{rt}", name=f"qt{rt}") for rt in range(NRT)]
        KT = [qkvp.tile([128, S], BF16, tag=f"kt{rt}", name=f"kt{rt}") for rt in range(NRT)]
        OT = [qkvp.tile([128, S], BF16, tag=f"ot{rt}", name=f"ot{rt}") for rt in range(NRT)]

        def emit_qk(store, w_sb, b_sb, rt, sq):
            ps = ps_mm.tile([128, CH], F32, tag="ps", name=f"psqk{rt}_{sq}")
            for kt in range(NK):
                nc.tensor.matmul(
                    ps[:],
                    w_sb[:, kt * DG + rt * 128: kt * DG + (rt + 1) * 128],
                    xT[:, sq * SQW + kt * CH: sq * SQW + (kt + 1) * CH],
                    start=(kt == 0), stop=(kt == NK - 1),
                )
            dst = store[rt][:, sq * CH:(sq + 1) * CH]
            if b_sb is not None:
                nc.scalar.activation(dst, ps[:], IDENT, bias=b_sb[:, rt:rt + 1])
            else:
                nc.vector.tensor_copy(dst, ps[:])

        def emit_v(st):
            sq, j = divmod(st, 4)
            ps = ps_mm.tile([128, CH], F32, tag="ps", name=f"psv{st}")
            for kt in range(NK):
                nc.tensor.matmul(
                    ps[:, :VW],
                    xT[:, sq * SQW + kt * CH + j * 128: sq * SQW + kt * CH + (j + 1) * 128],
                    wv_sb[:, kt * VW:(kt + 1) * VW],
                    start=(kt == 0), stop=(kt == NK - 1),
                )
            for hl in range(HG):
                nc.vector.tensor_copy(
                    V[st][:, hl * (HD + 1): hl * (HD + 1) + HD],
                    ps[:, hl * (HD + 1): hl * (HD + 1) + HD],
                )

        # ---- QKV for chunk 0 up-front; chunks 1-3 become fill units that are
        # interleaved into the attention loop (PE never idles on exp waits)
        for rt in range(NRT):
            emit_qk(QT, wq_sb, bq_sb, rt, 0)
            emit_qk(KT, wk_sb, bk_sb, rt, 0)
        for st in range(4):
            emit_v(st)

        fills = {ch: deque() for ch in range(NCH)}
        for sq in range(1, NCH):
            fq = fills[sq - 1]
            fq.append(lambda sq=sq: emit_qk(QT, wq_sb, bq_sb, 0, sq))
            fq.append(lambda sq=sq: emit_qk(KT, wk_sb, bk_sb, 0, sq))
            fq.append(lambda sq=sq: emit_v(sq * 4 + 0))
            fq.append(lambda sq=sq: emit_qk(QT, wq_sb, bq_sb, 1, sq))
            fq.append(lambda sq=sq: emit_qk(KT, wk_sb, bk_sb, 1, sq))
            fq.append(lambda sq=sq: emit_v(sq * 4 + 1))
            fq.append(lambda sq=sq: emit_v(sq * 4 + 2))
            fq.append(lambda sq=sq: emit_v(sq * 4 + 3))

        # ---- collective buffers: per 512-token chunk, AllGather the quad's
        # bf16 head outputs [256 ch, 512 tok] -> [1024 ch, 512 tok]; each core
        # then reads back only ITS OWN 128-token column slice (dynamic offset
        # g*128 from the per-core "gcol" input).
        ag_in = [dram.tile([DG, CH], BF16, tag=f"agi{c}", name=f"ag_in{c}")
                 for c in range(NCH)]
        ag_out = [dram.tile([HG * DG, CH], BF16, tag=f"ago{c}", name=f"ag_out{c}")
                  for c in range(NCH)]

        # gc load rides behind the xT loads — it's only needed ~100us in
        gc_sb = wpool.tile([1, 1], mybir.dt.uint32, tag="gc", name="gc_sb")
        nc.scalar.dma_start(gc_sb[:], gc_d[:])

        def emit_ag_in(ch, rt):
            nc.sync.dma_start(
                ag_in[ch][rt * 128:(rt + 1) * 128, :],
                OT[rt][:, ch * CH:(ch + 1) * CH],
            )

        def emit_a2a(ch):
            nc.gpsimd.collective_compute(
                "AllGather",
                mybir.AluOpType.bypass,
                replica_groups=[[0, 1, 2, 3], [4, 5, 6, 7]],
                ins=[ag_in[ch].opt()],
                outs=[ag_out[ch].opt()],
            )

        otf_tiles = {}

        # otf readback rides the gpsimd (SWDGE) queue: emitted right after the
        # NEXT chunk's AG trigger, its wait on AG(ch)-done never heads-of-line
        # blocks the SP queue or the broadcasts
        gcol_gp = None

        def emit_otf(ch):
            nonlocal gcol_gp
            if gcol_gp is None:
                _r = nc.gpsimd.alloc_register("gcol_gp_reg")
                nc.gpsimd.reg_load(_r, gc_sb[0:1, 0:1])
                gcol_gp = nc.gpsimd.snap(_r, donate=True, min_val=0, max_val=(HG - 1) * 128)
            otf = otfp.tile([128, NK * 128], BF16, tag="otf", name=f"otf{ch}")
            src = ag_out[ch][:, bass_ds(gcol_gp, 128)]
            nc.gpsimd.dma_start(
                otf[:].rearrange("p (kt t) -> p kt t", kt=NK),
                src.rearrange("(kt p) t -> p kt t", kt=NK),
            )
            otf_tiles[ch] = otf

        def emit_cproj(ch):
            otf = otf_tiles[ch]
            for n in range(2):
                po = ps_mm.tile([128, CH], F32, tag="ps", name=f"po{ch}_{n}")
                for kt in range(NK):
                    nc.tensor.matmul(
                        po[:],
                        otf[:, kt * 128:(kt + 1) * 128],
                        wp_sb[:, kt * D + n * CH: kt * D + (n + 1) * CH],
                        start=(kt == 0), stop=(kt == NK - 1),
                    )
                ob = outp.tile([128, CH], F32, tag="ob", name=f"ob{ch}_{n}")
                if has_bp:
                    nc.vector.tensor_add(ob[:], po[:], bp_sb[:, n * CH:(n + 1) * CH])
                else:
                    nc.vector.tensor_copy(ob[:], po[:])
                nc.sync.dma_start(out_d[ch * 128:(ch + 1) * 128, n * CH:(n + 1) * CH], ob[:])

        def normalize(ot_ps, rt, off, ch):
            den = nrm.tile([1, CH], F32, tag="den", name=f"den{ch}_{rt}_{off}")
            nc.vector.tensor_copy(den[:], ot_ps[64:65, :])
            rden = nrm.tile([1, CH], F32, tag="rden", name=f"rden{ch}_{rt}_{off}")
            nc.vector.reciprocal_approx_fast(rden[:], den[:])
            rbc = nrm.tile([64, CH], F32, tag="rbc", name=f"rbc{ch}_{rt}_{off}")
            nc.gpsimd.partition_broadcast(rbc[:], rden[:], channels=64)
            dst = OT[rt][off:off + 64, ch * CH:(ch + 1) * CH]
            nc.vector.tensor_mul(dst, ot_ps[0:64, :], rbc[:])
            if has_bv:
                nc.vector.tensor_scalar_add(dst, dst, bv_sb[off:off + 64, rt:rt + 1])

        # ---- attention chunks, with QKV fill units woven in.  cproj for
        # chunk ch-1 (otf readback + matmuls) is emitted right after chunk
        # ch's AllGather trigger: by then AG(ch-1) has completed, so neither
        # the PE FIFO nor the SP DMA queue ever blocks on a collective
        # mid-stream — only the final chunk's AG is exposed.
        for ch in range(NCH):
            nkt = 4 * (ch + 1)
            nsteps = 2 * nkt
            fq = fills[ch]
            n_fill = len(fq)
            emitted = 0
            step = 0
            for rt in range(NRT):
                ot_a = ps_ot.tile([65, CH], F32, tag="ot", name=f"ota{ch}_{rt}")
                ot_b = ps_ot.tile([65, CH], F32, tag="ot", name=f"otb{ch}_{rt}")
                for kt in range(nkt):
                    st_ps = ps_st.tile([128, 2 * CH], F32, tag="st", name=f"st{ch}_{rt}_{kt}")
                    for half, off in ((0, 0), (1, 64)):
                        nc.tensor.matmul(
                            st_ps[:, half * CH:(half + 1) * CH],
                            KT[rt][off:off + 64, kt * 128:(kt + 1) * 128],
                            QT[rt][off:off + 64, ch * CH:(ch + 1) * CH],
                            start=True, stop=True,
                        )
                    st_sb = stp.tile([128, 2 * CH], BF16, tag="stsb", name=f"se{ch}_{rt}_{kt}")
                    nc.scalar.activation(st_sb[:], st_ps[:], EXP, scale=0.125)
                    d = kt - 4 * ch
                    if d >= 0:
                        for half in range(2):
                            blk = st_sb[:, half * CH + d * 128: half * CH + (d + 1) * 128]
                            nc.vector.tensor_mul(blk, blk, mk_sb[:, 0:128])
                    # fill units between the score matmuls and the AV matmuls
                    while emitted * nsteps < (step + 1) * n_fill:
                        fq.popleft()()
                        emitted += 1
                    lo = max(d, 0) * 128 if d > 0 else 0
                    for half, ot_ps in ((0, ot_a), (1, ot_b)):
                        nc.tensor.matmul(
                            ot_ps[:, lo:],
                            V[kt][:, (rt * 2 + half) * (HD + 1):(rt * 2 + half + 1) * (HD + 1)],
                            st_sb[:, half * CH + lo:(half + 1) * CH],
                            start=(kt == 0), stop=(kt == nkt - 1),
                        )
                    step += 1
                normalize(ot_a, rt, 0, ch)
                normalize(ot_b, rt, 64, ch)
                emit_ag_in(ch, rt)
            emit_a2a(ch)
            if ch > 0:
                emit_otf(ch - 1)
            if ch > 1:
                emit_cproj(ch - 2)
        emit_otf(NCH - 1)
        emit_cproj(NCH - 2)
        emit_cproj(NCH - 1)

    nc.compile()
    return nc


_prog_cache = {}


def _get_prog(has_bv, has_bp, has_bqk):
    key = (has_bv, has_bp, has_bqk)
    if key not in _prog_cache:
        _prog_cache[key] = _build(*key)
    return _prog_cache[key]


def _pack_kmajor(w):
    # [D, X] -> [128, NK*X] with col = kt*X + x, so sbuf[:, kt*X+x] = w[kt*128+p, x]
    dd, x = w.shape
    return np.ascontiguousarray(
        w.reshape(NK, 128, x).transpose(1, 0, 2).reshape(128, NK * x)
    )


def _prepare(x, w_attn, b_attn, w_proj, b_proj):
    x = np.asarray(x, dtype=np.float32)
    w_attn = np.asarray(w_attn, dtype=np.float32)
    b_attn = np.asarray(b_attn, dtype=np.float32)
    w_proj = np.asarray(w_proj, dtype=np.float32)
    b_proj = np.asarray(b_proj, dtype=np.float32)

    has_bv = bool(np.any(b_attn[2 * D:]))
    has_bp = bool(np.any(b_proj))
    has_bqk = bool(np.any(b_attn[:2 * D]))
    nc = _get_prog(has_bv, has_bp, has_bqk)

    ii = np.arange(128)[:, None]
    jj = np.arange(128)[None, :]
    masks = (jj >= ii).astype(np.float32).astype(ml_dtypes.bfloat16)

    # per-batch activations, transposed + packed to the SBUF layout
    # (col = sq*4096 + kt*512 + t), shared across the 4 group-cores
    xT_b = [
        np.ascontiguousarray(
            x[b].reshape(NCH, CH, NK, 128).transpose(3, 0, 2, 1).reshape(128, NK * S)
        ).astype(ml_dtypes.bfloat16)
        for b in range(B)
    ]
    # per-group packed weights (shared across the 2 batch-cores)
    wq_g, wk_g, wv_g, bq_g, bk_g, bv_g = [], [], [], [], [], []
    for g in range(HG):
        q0 = g * DG
        k0 = D + g * DG
        v0 = 2 * D + g * DG
        wv_ext = np.zeros((D, VW), dtype=np.float32)
        for hl in range(HG):
            wv_ext[:, hl * (HD + 1):hl * (HD + 1) + HD] = \
                w_attn[:, v0 + hl * HD: v0 + (hl + 1) * HD]
        wq_g.append(_pack_kmajor(w_attn[:, q0:q0 + DG]).astype(ml_dtypes.bfloat16))
        wk_g.append(_pack_kmajor(w_attn[:, k0:k0 + DG]).astype(ml_dtypes.bfloat16))
        wv_g.append(_pack_kmajor(wv_ext).astype(ml_dtypes.bfloat16))
        bq_g.append(np.ascontiguousarray(b_attn[q0:q0 + DG, None]))
        bk_g.append(np.ascontiguousarray(b_attn[k0:k0 + DG, None]))
        bv_g.append(np.ascontiguousarray(b_attn[v0:v0 + DG, None]))
    wp_pack = _pack_kmajor(w_proj).astype(ml_dtypes.bfloat16)
    bp_tile = np.broadcast_to(b_proj, (128, D)).astype(np.float32)

    in_maps = []
    for c in range(N_CORES):
        b, g = divmod(c, 4)
        in_maps.append({
            "xT": xT_b[b],
            "wq": wq_g[g],
            "wk": wk_g[g],
            "wv": wv_g[g],
            "wp": wp_pack,
            "bq": bq_g[g],
            "bk": bk_g[g],
            "bv": bv_g[g],
            "bp": bp_tile,
            "masks": masks,
            "gcol": np.array([[g * 128]], dtype=np.uint32),
        })
    return nc, in_maps


def _assemble(results):
    out = np.empty((B, S, D), dtype=np.float32)
    for c in range(N_CORES):
        b, g = divmod(c, 4)
        o = results[c]["out"]
        for ch in range(NCH):
            tok = ch * CH + g * 128
            out[b, tok:tok + 128, :] = o[ch * 128:(ch + 1) * 128, :]
    return out


def kernel(x, w_attn, b_attn, w_proj, b_proj):
    nc, in_maps = _prepare(x, w_attn, b_attn, w_proj, b_proj)
    res = run_bass_kernel_spmd(nc, in_maps, list(range(N_CORES)))
    return _assemble(res.results)


# revision 26
# speedup vs baseline: 1.6517x; 1.0005x over previous
"""GPT2 eager causal attention (B=2, S=2048, D=1024, H=16, HD=64) on 8 TRN2 NeuronCores.

Sharding (data + head/tensor parallel): core c -> (batch b = c//4, head-group
g = c%4), 4 heads per group.  Token ownership for the output: core (b, g) owns
token rows [ch*512 + g*128, +128) of batch b, for each 512-token chunk ch.

v2 pipeline (vs the RS baseline):
  - x is transposed on the HOST -> xT [D, S]; no transpose-DMAs on device.
  - weights are host-packed into SBUF layout -> one large DMA each, spread
    across the SP/ACT/gpsimd DMA queues so nothing serializes at startup.
  - QT/KT [256, S] and V [S, 260] (ones-column per head for the free softmax
    denominator) as in the baseline, but QKV compute for token-chunk sq>=1 is
    software-pipelined INTO the attention loop of chunk sq-1 to keep PE dense
    (HAM stays warm) and to overlap ACT-exp with PE matmuls.
  - scores: the two heads of a KT row-tile run as CONCURRENT 64-contract
    matmuls on PE row-groups 0-63/64-127 writing adjacent PSUM banks; one
    batched EXP (FD=1024) covers both.  Diagonal tiles exp the full tile
    (garbage prefix cols are simply never streamed by the AV matmul) and
    triangular-mask only the 128-wide diagonal block.
  - softmax normalize: DVE reciprocal of the ones-row directly from PSUM,
    gpsimd partition_broadcast (instead of a PE rank-1 matmul), DVE multiply.
  - c_proj: per 512-token chunk, AllToAll over the quad exchanges bf16 head
    outputs so each core receives ALL 1024 channels for ITS OWN 128-token
    slice; c_proj then contracts the full D with the full w_proj.  ~4x less
    wire than the fp32 ReduceScatter and a much shorter serial tail.
"""
from collections import deque
from contextlib import ExitStack

import ml_dtypes
import numpy as np

import concourse.bacc as bacc
import concourse.mybir as mybir
import concourse.tile as tile
from concourse.bass import ds as bass_ds
from concourse.bass_utils import run_bass_kernel_spmd

F32 = mybir.dt.float32
BF16 = mybir.dt.bfloat16
U16 = mybir.dt.uint16

B, S, D, H, HD = 2, 2048, 1024, 16, 64
N_CORES = 8
HG = 4               # heads per group
DG = HG * HD         # 256 q/k channels per group
VW = HG * (HD + 1)   # 260: 64 v-cols + 1 ones-col per head
NK = D // 128        # 8 contraction tiles over d
NS = S // 128        # 16 token tiles
CH = 512             # q-chunk (one PSUM bank of fp32)
NCH = S // CH        # 4
NRT = DG // 128      # 2 channel row-tiles (head pairs) per group
SQW = NK * CH        # 4096: xT sbuf columns per token chunk

EXP = mybir.ActivationFunctionType.Exp
IDENT = mybir.ActivationFunctionType.Identity


def _build(has_bv: bool, has_bp: bool, has_bqk: bool):
    nc = bacc.Bacc("TRN2", target_bir_lowering=False, debug=False, num_devices=N_CORES)

    xT_d = nc.dram_tensor("xT", [128, NK * S], BF16, kind="ExternalInput").ap()
    # wq | wk | wv | mask packed back-to-back: one startup DMA
    WQKV = 2 * NK * DG + NK * VW + 128
    wqkv_d = nc.dram_tensor("wqkv", [128, WQKV], BF16, kind="ExternalInput").ap()
    wp_d = nc.dram_tensor("wp", [128, NK * D], BF16, kind="ExternalInput").ap()
    bq_d = nc.dram_tensor("bq", [DG, 1], F32, kind="ExternalInput").ap()
    bk_d = nc.dram_tensor("bk", [DG, 1], F32, kind="ExternalInput").ap()
    bv_d = nc.dram_tensor("bv", [DG, 1], F32, kind="ExternalInput").ap()
    bp_d = nc.dram_tensor("bp", [128, D], F32, kind="ExternalInput").ap()
    mk_d = nc.dram_tensor("masks", [128, 128], BF16, kind="ExternalInput").ap()
    gc_d = nc.dram_tensor("gcol", [1, 1], mybir.dt.uint32, kind="ExternalInput").ap()
    out_d = nc.dram_tensor("out", [NCH * 128, D], F32, kind="ExternalOutput").ap()

    with ExitStack() as ctx:
        tc = ctx.enter_context(tile.TileContext(nc))
        wpool = ctx.enter_context(tc.tile_pool(name="w", bufs=1))
        qkvp = ctx.enter_context(tc.tile_pool(name="qkv", bufs=1))
        stp = ctx.enter_context(tc.tile_pool(name="stx", bufs=3))
        nrm = ctx.enter_context(tc.tile_pool(name="nrm", bufs=3))
        otfp = ctx.enter_context(tc.tile_pool(name="otf", bufs=2))
        outp = ctx.enter_context(tc.tile_pool(name="outp", bufs=3))
        ps_st = ctx.enter_context(tc.tile_pool(name="psst", bufs=2, space="PSUM"))
        ps_ot = ctx.enter_context(tc.tile_pool(name="psot", bufs=2, space="PSUM"))
        ps_mm = ctx.enter_context(tc.tile_pool(name="psmm", bufs=2, space="PSUM"))
        dram = ctx.enter_context(tc.tile_pool(name="dram", bufs=1, space="DRAM"))

        # ---- tiny exp at t=0 forces the ACT table load to overlap the DMAs
        dmy = wpool.tile([128, 1], F32, tag="dmy", name="dmy")
        dmo = wpool.tile([128, 1], F32, tag="dmo", name="dmo")
        nc.vector.memset(dmy[:], 0.0)
        nc.scalar.activation(dmo[:], dmy[:], EXP, scale=1.0)

        # ---- weights: wq|wk|wv|mask as ONE contiguous DMA on the ACT queue,
        # wp (only needed ~100us in) on the gpsimd queue
        wqkv_sb = wpool.tile([128, WQKV], BF16, tag="wqkv", name="wqkv_sb")
        wp_sb = wpool.tile([128, NK * D], BF16, tag="wp", name="wp_sb")
        nc.scalar.dma_start(wqkv_sb[:], wqkv_d[:])
        nc.gpsimd.dma_start(wp_sb[:], wp_d[:])
        wq_sb = wqkv_sb[:, 0:NK * DG]
        wk_sb = wqkv_sb[:, NK * DG:2 * NK * DG]
        wv_sb = wqkv_sb[:, 2 * NK * DG:2 * NK * DG + NK * VW]
        mk_sb = wqkv_sb[:, 2 * NK * DG + NK * VW:WQKV]
        bq_sb = bk_sb = bv_sb = bp_sb = None
        if has_bqk:
            bq_sb = wpool.tile([128, NRT], F32, tag="bq", name="bq_sb")
            bk_sb = wpool.tile([128, NRT], F32, tag="bk", name="bk_sb")
            for rt in range(NRT):
                nc.scalar.dma_start(bq_sb[:, rt:rt + 1], bq_d[rt * 128:(rt + 1) * 128, :])
                nc.scalar.dma_start(bk_sb[:, rt:rt + 1], bk_d[rt * 128:(rt + 1) * 128, :])
        if has_bv:
            bv_sb = wpool.tile([128, NRT], F32, tag="bv", name="bv_sb")
            for rt in range(NRT):
                nc.scalar.dma_start(bv_sb[:, rt:rt + 1], bv_d[rt * 128:(rt + 1) * 128, :])
        if has_bp:
            bp_sb = wpool.tile([128, D], F32, tag="bp", name="bp_sb")
            nc.scalar.dma_start(bp_sb[:], bp_d[:])

        # ---- V tiles exist up-front; set the whole tile to bf16 1.0 now so
        # the later per-head copies only write the 64 v-columns
        V = []
        for st in range(NS):
            vt = qkvp.tile([128, VW], BF16, tag=f"v{st}", name=f"v{st}")
            nc.vector.memset(vt[:].bitcast(U16), 0x3F80)
            V.append(vt)

        # ---- xT loads: host pre-packed to SBUF layout (col = sq*4096 + kt*512
        # + t), so each chunk is one fully-contiguous 1MB DMA
        xT = qkvp.tile([128, NK * S], BF16, tag="xT", name="xT")
        # sq0 lands in two halves so the first Q chain starts ~2us sooner
        nc.sync.dma_start(xT[:, 0:SQW // 2], xT_d[:, 0:SQW // 2])
        nc.sync.dma_start(xT[:, SQW // 2:SQW], xT_d[:, SQW // 2:SQW])
        for sq in range(1, NCH):
            nc.sync.dma_start(
                xT[:, sq * SQW:(sq + 1) * SQW], xT_d[:, sq * SQW:(sq + 1) * SQW]
            )

        QT = [qkvp.tile([128, S], BF16, tag=f"qt{rt}", name=f"qt{rt}") for rt in range(NRT)]
        KT = [qkvp.tile([128, S], BF16, tag=f"kt{rt}", name=f"kt{rt}") for rt in range(NRT)]
        OT = [qkvp.tile([128, S], BF16, tag=f"ot{rt}", name=f"ot{rt}") for rt in range(NRT)]

        def emit_qk(store, w_sb, b_sb, rt, sq):
            ps = ps_mm.tile([128, CH], F32, tag="ps", name=f"psqk{rt}_{sq}")
            for kt in range(NK):
                nc.tensor.matmul(
                    ps[:],
                    w_sb[:, kt * DG + rt * 128: kt * DG + (rt + 1) * 128],
                    xT[:, sq * SQW + kt * CH: sq * SQW + (kt + 1) * CH],
                    start=(kt == 0), stop=(kt == NK - 1),
                )
            dst = store[rt][:, sq * CH:(sq + 1) * CH]
            if b_sb is not None:
                nc.scalar.activation(dst, ps[:], IDENT, bias=b_sb[:, rt:rt + 1])
            else:
                nc.vector.tensor_copy(dst, ps[:])

        def emit_v(st):
            sq, j = divmod(st, 4)
            ps = ps_mm.tile([128, CH], F32, tag="ps", name=f"psv{st}")
            for kt in range(NK):
                nc.tensor.matmul(
                    ps[:, :VW],
                    xT[:, sq * SQW + kt * CH + j * 128: sq * SQW + kt * CH + (j + 1) * 128],
                    wv_sb[:, kt * VW:(kt + 1) * VW],
                    start=(kt == 0), stop=(kt == NK - 1),
                )
            for hl in range(HG):
                nc.vector.tensor_copy(
                    V[st][:, hl * (HD + 1): hl * (HD + 1) + HD],
                    ps[:, hl * (HD + 1): hl * (HD + 1) + HD],
                )

        # ---- QKV for chunk 0 up-front; chunks 1-3 become fill units that are
        # interleaved into the attention loop (PE never idles on exp waits)
        for rt in range(NRT):
            emit_qk(QT, wq_sb, bq_sb, rt, 0)
            emit_qk(KT, wk_sb, bk_sb, rt, 0)
        for st in range(4):
            emit_v(st)

        fills = {ch: deque() for ch in range(NCH)}
        for sq in range(1, NCH):
            fq = fills[sq - 1]
            fq.append(lambda sq=sq: emit_qk(QT, wq_sb, bq_sb, 0, sq))
            fq.append(lambda sq=sq: emit_qk(KT, wk_sb, bk_sb, 0, sq))
            fq.append(lambda sq=sq: emit_v(sq * 4 + 0))
            fq.append(lambda sq=sq: emit_qk(QT, wq_sb, bq_sb, 1, sq))
            fq.append(lambda sq=sq: emit_qk(KT, wk_sb, bk_sb, 1, sq))
            fq.append(lambda sq=sq: emit_v(sq * 4 + 1))
            fq.append(lambda sq=sq: emit_v(sq * 4 + 2))
            fq.append(lambda sq=sq: emit_v(sq * 4 + 3))

        # ---- collective buffers: per 512-token chunk, AllGather the quad's
        # bf16 head outputs [256 ch, 512 tok] -> [1024 ch, 512 tok]; each core
        # then reads back only ITS OWN 128-token column slice (dynamic offset
        # g*128 from the per-core "gcol" input).
        ag_in = [dram.tile([DG, CH], BF16, tag=f"agi{c}", name=f"ag_in{c}")
                 for c in range(NCH)]
        ag_out = [dram.tile([HG * DG, CH], BF16, tag=f"ago{c}", name=f"ag_out{c}")
                  for c in range(NCH)]

        # gc load rides behind the xT loads — it's only needed ~100us in
        gc_sb = wpool.tile([1, 1], mybir.dt.uint32, tag="gc", name="gc_sb")
        nc.scalar.dma_start(gc_sb[:], gc_d[:])

        def emit_ag_in(ch, rt):
            nc.sync.dma_start(
                ag_in[ch][rt * 128:(rt + 1) * 128, :],
                OT[rt][:, ch * CH:(ch + 1) * CH],
            )

        def emit_a2a(ch):
            nc.gpsimd.collective_compute(
                "AllGather",
                mybir.AluOpType.bypass,
                replica_groups=[[0, 1, 2, 3], [4, 5, 6, 7]],
                ins=[ag_in[ch].opt()],
                outs=[ag_out[ch].opt()],
            )

        otf_tiles = {}

        # otf readback rides the gpsimd (SWDGE) queue: emitted right after the
        # NEXT chunk's AG trigger, its wait on AG(ch)-done never heads-of-line
        # blocks the SP queue or the broadcasts
        gcol_gp = None

        def emit_otf(ch):
            nonlocal gcol_gp
            if gcol_gp is None:
                _r = nc.gpsimd.alloc_register("gcol_gp_reg")
                nc.gpsimd.reg_load(_r, gc_sb[0:1, 0:1])
                gcol_gp = nc.gpsimd.snap(_r, donate=True, min_val=0, max_val=(HG - 1) * 128)
            otf = otfp.tile([128, NK * 128], BF16, tag="otf", name=f"otf{ch}")
            src = ag_out[ch][:, bass_ds(gcol_gp, 128)]
            nc.gpsimd.dma_start(
                otf[:].rearrange("p (kt t) -> p kt t", kt=NK),
                src.rearrange("(kt p) t -> p kt t", kt=NK),
            )
            otf_tiles[ch] = otf

        def emit_cproj(ch):
            otf = otf_tiles[ch]
            for n in range(2):
                po = ps_mm.tile([128, CH], F32, tag="ps", name=f"po{ch}_{n}")
                for kt in range(NK):
                    nc.tensor.matmul(
                        po[:],
                        otf[:, kt * 128:(kt + 1) * 128],
                        wp_sb[:, kt * D + n * CH: kt * D + (n + 1) * CH],
                        start=(kt == 0), stop=(kt == NK - 1),
                    )
                ob = outp.tile([128, CH], F32, tag="ob", name=f"ob{ch}_{n}")
                if has_bp:
                    nc.vector.tensor_add(ob[:], po[:], bp_sb[:, n * CH:(n + 1) * CH])
                else:
                    nc.vector.tensor_copy(ob[:], po[:])
                nc.sync.dma_start(out_d[ch * 128:(ch + 1) * 128, n * CH:(n + 1) * CH], ob[:])

        def normalize(ot_ps, rt, off, ch):
            den = nrm.tile([1, CH], F32, tag="den", name=f"den{ch}_{rt}_{off}")
            nc.vector.tensor_copy(den[:], ot_ps[64:65, :])
            rden = nrm.tile([1, CH], F32, tag="rden", name=f"rden{ch}_{rt}_{off}")
            nc.vector.reciprocal_approx_fast(rden[:], den[:])
            rbc = nrm.tile([64, CH], F32, tag="rbc", name=f"rbc{ch}_{rt}_{off}")
            nc.gpsimd.partition_broadcast(rbc[:], rden[:], channels=64)
            dst = OT[rt][off:off + 64, ch * CH:(ch + 1) * CH]
            nc.vector.tensor_mul(dst, ot_ps[0:64, :], rbc[:])
            if has_bv:
                nc.vector.tensor_scalar_add(dst, dst, bv_sb[off:off + 64, rt:rt + 1])

        # ---- attention chunks, with QKV fill units woven in.  cproj for
        # chunk ch-1 (otf readback + matmuls) is emitted right after chunk
        # ch's AllGather trigger: by then AG(ch-1) has completed, so neither
        # the PE FIFO nor the SP DMA queue ever blocks on a collective
        # mid-stream — only the final chunk's AG is exposed.
        for ch in range(NCH):
            nkt = 4 * (ch + 1)
            nsteps = 2 * nkt
            fq = fills[ch]
            n_fill = len(fq)
            emitted = 0
            step = 0
            for rt in range(NRT):
                ot_a = ps_ot.tile([65, CH], F32, tag="ot", name=f"ota{ch}_{rt}")
                ot_b = ps_ot.tile([65, CH], F32, tag="ot", name=f"otb{ch}_{rt}")
                for kt in range(nkt):
                    st_ps = ps_st.tile([128, 2 * CH], F32, tag="st", name=f"st{ch}_{rt}_{kt}")
                    for half, off in ((0, 0), (1, 64)):
                        nc.tensor.matmul(
                            st_ps[:, half * CH:(half + 1) * CH],
                            KT[rt][off:off + 64, kt * 128:(kt + 1) * 128],
                            QT[rt][off:off + 64, ch * CH:(ch + 1) * CH],
                            start=True, stop=True,
                        )
                    st_sb = stp.tile([128, 2 * CH], BF16, tag="stsb", name=f"se{ch}_{rt}_{kt}")
                    nc.scalar.activation(st_sb[:], st_ps[:], EXP, scale=0.125)
                    d = kt - 4 * ch
                    if d >= 0:
                        for half in range(2):
                            blk = st_sb[:, half * CH + d * 128: half * CH + (d + 1) * 128]
                            nc.vector.tensor_mul(blk, blk, mk_sb[:, 0:128])
                    # fill units between the score matmuls and the AV matmuls
                    while emitted * nsteps < (step + 1) * n_fill:
                        fq.popleft()()
                        emitted += 1
                    lo = max(d, 0) * 128 if d > 0 else 0
                    for half, ot_ps in ((0, ot_a), (1, ot_b)):
                        nc.tensor.matmul(
                            ot_ps[:, lo:],
                            V[kt][:, (rt * 2 + half) * (HD + 1):(rt * 2 + half + 1) * (HD + 1)],
                            st_sb[:, half * CH + lo:(half + 1) * CH],
                            start=(kt == 0), stop=(kt == nkt - 1),
                        )
                    step += 1
                normalize(ot_a, rt, 0, ch)
                normalize(ot_b, rt, 64, ch)
                emit_ag_in(ch, rt)
            emit_a2a(ch)
            if ch > 0:
                emit_otf(ch - 1)
            if ch > 1:
                emit_cproj(ch - 2)
        emit_otf(NCH - 1)
        emit_cproj(NCH - 2)
        emit_cproj(NCH - 1)

    nc.compile()
    return nc


_prog_cache = {}


def _get_prog(has_bv, has_bp, has_bqk):
    key = (has_bv, has_bp, has_bqk)
    if key not in _prog_cache:
        _prog_cache[key] = _build(*key)
    return _prog_cache[key]


def _pack_kmajor(w):
    # [D, X] -> [128, NK*X] with col = kt*X + x, so sbuf[:, kt*X+x] = w[kt*128+p, x]
    dd, x = w.shape
    return np.ascontiguousarray(
        w.reshape(NK, 128, x).transpose(1, 0, 2).reshape(128, NK * x)
    )


def _prepare(x, w_attn, b_attn, w_proj, b_proj):
    x = np.asarray(x, dtype=np.float32)
    w_attn = np.asarray(w_attn, dtype=np.float32)
    b_attn = np.asarray(b_attn, dtype=np.float32)
    w_proj = np.asarray(w_proj, dtype=np.float32)
    b_proj = np.asarray(b_proj, dtype=np.float32)

    has_bv = bool(np.any(b_attn[2 * D:]))
    has_bp = bool(np.any(b_proj))
    has_bqk = bool(np.any(b_attn[:2 * D]))
    nc = _get_prog(has_bv, has_bp, has_bqk)

    ii = np.arange(128)[:, None]
    jj = np.arange(128)[None, :]
    masks = (jj >= ii).astype(np.float32).astype(ml_dtypes.bfloat16)

    # per-batch activations, transposed + packed to the SBUF layout
    # (col = sq*4096 + kt*512 + t), shared across the 4 group-cores
    xT_b = [
        np.ascontiguousarray(
            x[b].reshape(NCH, CH, NK, 128).transpose(3, 0, 2, 1).reshape(128, NK * S)
        ).astype(ml_dtypes.bfloat16)
        for b in range(B)
    ]
    # per-group packed weights (shared across the 2 batch-cores)
    wq_g, wk_g, wv_g, bq_g, bk_g, bv_g = [], [], [], [], [], []
    for g in range(HG):
        q0 = g * DG
        k0 = D + g * DG
        v0 = 2 * D + g * DG
        wv_ext = np.zeros((D, VW), dtype=np.float32)
        for hl in range(HG):
            wv_ext[:, hl * (HD + 1):hl * (HD + 1) + HD] = \
                w_attn[:, v0 + hl * HD: v0 + (hl + 1) * HD]
        wqkv = np.concatenate(
            [
                _pack_kmajor(w_attn[:, q0:q0 + DG]),
                _pack_kmajor(w_attn[:, k0:k0 + DG]),
                _pack_kmajor(wv_ext),
                (jj >= ii).astype(np.float32),
            ],
            axis=1,
        )
        wq_g.append(np.ascontiguousarray(wqkv).astype(ml_dtypes.bfloat16))
        bq_g.append(np.ascontiguousarray(b_attn[q0:q0 + DG, None]))
        bk_g.append(np.ascontiguousarray(b_attn[k0:k0 + DG, None]))
        bv_g.append(np.ascontiguousarray(b_attn[v0:v0 + DG, None]))
    wp_pack = _pack_kmajor(w_proj).astype(ml_dtypes.bfloat16)
    bp_tile = np.broadcast_to(b_proj, (128, D)).astype(np.float32)

    in_maps = []
    for c in range(N_CORES):
        b, g = divmod(c, 4)
        in_maps.append({
            "xT": xT_b[b],
            "wqkv": wq_g[g],
            "wp": wp_pack,
            "bq": bq_g[g],
            "bk": bk_g[g],
            "bv": bv_g[g],
            "bp": bp_tile,
            "masks": masks,
            "gcol": np.array([[g * 128]], dtype=np.uint32),
        })
    return nc, in_maps


def _assemble(results):
    out = np.empty((B, S, D), dtype=np.float32)
    for c in range(N_CORES):
        b, g = divmod(c, 4)
        o = results[c]["out"]
        for ch in range(NCH):
            tok = ch * CH + g * 128
            out[b, tok:tok + 128, :] = o[ch * 128:(ch + 1) * 128, :]
    return out


def kernel(x, w_attn, b_attn, w_proj, b_proj):
    nc, in_maps = _prepare(x, w_attn, b_attn, w_proj, b_proj)
    res = run_bass_kernel_spmd(nc, in_maps, list(range(N_CORES)))
    return _assemble(res.results)
